# revision 50
# baseline (speedup 1.0000x reference)
"""Trainium2 Bass kernel for MinimalEventMamba.

kernel(**inputs) takes FULL inputs (as from setup_inputs()) and returns the
FULL (4, 10, 64, 64) float32 output. Internally: batch-parallel across 8
NeuronCores (4 batches x2 replicated, state-split across the pair), one SPMD
Bass kernel launch, host assembles the output.

Per-core layout: channel-on-partition, L=4096 on free dim.
- encoder conv as im2col matmul (own batch only; BN stats via 8-way AllReduce)
- mamba trunk: depthwise conv folded into in_proj taps (shifted matmuls,
  PSUM accumulate), dt_w folded into x_proj, Silu/Softplus computed as single
  fused Act ops on PSUM eviction, selective scan via tensor_tensor_scan on
  DVE per state dim, B/C rows broadcast across partitions via DRAM-bounce
  DMA, hc-multiplies offloaded to GpSimd, residual updates on GpSimd.
- per-layer y AllReduce split into two halves pipelined behind the scan
  phase (half-0 collective runs under half-1 scans; half-1 collective runs
  under the next layer's chunk work).
- decoder: dec1 duplicated per pair (cross-batch BN stats via AllReduce),
  dec2 split by output channel across the pair (5+5).
"""
import sys
import types

sys.path.insert(0, "/opt/trn_rl_repo")
sys.path.insert(0, "/opt/trn_rl_repo/concourse")
try:
    from antenv import axon_hooks  # noqa: F401
except ImportError:
    try:
        from trn_agent_boot.trn_boot import _ntff_profile_via_ctypes
        _m = types.ModuleType("antenv.axon_hooks")
        _h = _ntff_profile_via_ctypes("/opt/axon/libaxon_pjrt.so")
        _m.get_axon_ntff_profile_hook = lambda: _h
        _m.set_axon_ntff_profile_hook = lambda h: None
        sys.modules["antenv.axon_hooks"] = _m
    except Exception:
        pass

from contextlib import ExitStack

import numpy as np
import ml_dtypes

import concourse.bass as bass
import concourse.tile as tile
from concourse import mybir
from concourse.bass_utils import run_bass_kernel_spmd
import bass_rust

F32 = mybir.dt.float32
F32R = mybir.dt.float32r
BF16 = mybir.dt.bfloat16
FP16 = mybir.dt.float16

NB, HD, NL, NF = 5, 64, 4, 10
DI, DS, DC, DTR = 128, 16, 4, 4
B, H, W = 4, 64, 64
L = H * W                     # 4096
PW = W + 2                    # padded row stride 66
PADL = PW * (H + 2) + 4       # padded spatial + guard cols (4360)
PBASE = 1 + PW + 1            # first interior col in padded layout
NCHUNK = 8                    # L / 512
CS = 512
LH = L // 2                   # half length (2048)
DSL = DS // 2                 # states per core (s-split across core pairs)
NFH = NF // 2                 # dec2 output channels per core

HC_ON_POOL = False            # GpSimd elementwise measured 4x slower than DVE
                              # 2x mode AND contends for SBUF ports with DVE


def split_excess_waits(nc, max_waits=1):
    """This container's walrus accepts only 1 sync wait per instruction;
    move overflow waits onto NOPs inserted before the offending op."""
    f = nc.m.functions[0]
    for bb in f.blocks:
        insts = bb.instructions
        i = 0
        while i < len(insts):
            inst = insts[i]
            si = inst.sync_info
            if si is not None and len(si.on_wait) > max_waits:
                waits = list(si.on_wait)
                si.on_wait = waits[-max_waits:]
                inst.sync_info = si
                overflow = waits[:-max_waits]
                eng = nc.engines[inst.engine]
                pos = i
                for j in range(0, len(overflow), max_waits):
                    nop = eng.nop(hint="splitw", nofuse=True)
                    nop_inst = nop.ins if hasattr(nop, "ins") else nop
                    for bb2 in f.blocks:
                        if any(x is nop_inst for x in bb2.instructions):
                            bb2.instructions[:] = [
                                x for x in bb2.instructions if x is not nop_inst
                            ]
                            break
                    nop_inst.sync_info = bass_rust.SyncInfo(
                        on_wait=overflow[j : j + max_waits], on_update=[]
                    )
                    insts.insert(pos, nop_inst)
                    pos += 1
                i = pos + 1
            else:
                i += 1


def build_kernel():
    nc = bass.Bass()
    dp = nc.declare_dram_parameter

    enc_in = dp("enc_im2col", [45, B * L], FP16, isOutput=False)
    enc_w2 = dp("enc_w2", [45, HD], FP16, isOutput=False)
    enc_g = dp("enc_g", [HD, 1], F32, isOutput=False)
    enc_be = dp("enc_be", [HD, 1], F32, isOutput=False)
    ip_tap = dp("ip_tap", [HD, NL * DC * DI], FP16, isOutput=False)
    ip_z = dp("ip_z", [HD, NL * DI], FP16, isOutput=False)
    conv_b = dp("conv_b", [DI, NL], F32, isOutput=False)
    wd_T = dp("wd_T", [DI, NL * DI], BF16, isOutput=False)
    bc_T = dp("bc_T", [DI, NL * 2 * DSL], BF16, isOutput=False)
    dt_b = dp("dt_b", [DI, NL], F32, isOutput=False)
    a_cols = dp("a_cols", [DI, NL * DSL], F32, isOutput=False)
    d_col = dp("d_col", [DI, NL], F32, isOutput=False)
    op_T = dp("op_T", [DI, NL * HD], BF16, isOutput=False)
    dec1_tap = dp("dec1_tap", [HD, 9 * HD], FP16, isOutput=False)
    dec1_g = dp("dec1_g", [HD, 1], F32, isOutput=False)
    dec1_be = dp("dec1_be", [HD, 1], F32, isOutput=False)
    dec2_tap = dp("dec2_tap", [HD, 9 * NFH], FP16, isOutput=False)
    dec2_b = dp("dec2_b", [NFH, 1], F32, isOutput=False)

    out_ext = dp("out", [NFH, L], BF16, isOutput=True)

    bc_dram = nc.dram_tensor("bc_dram", [NL, 2 * DSL, L], BF16)
    y_in = [nc.dram_tensor(f"y_in{h}", [HD, LH], BF16) for h in range(2)]
    y_out = [nc.dram_tensor(f"y_out{h}", [HD, LH], BF16) for h in range(2)]
    cc_in = nc.dram_tensor("cc_in", [HD, 2], F32)
    cc_out = nc.dram_tensor("cc_out", [HD, 2], F32)

    ctx = ExitStack()
    with ctx:
        tc = ctx.enter_context(tile.TileContext(nc))
        const = ctx.enter_context(tc.tile_pool(name="const", bufs=1))
        persist = ctx.enter_context(tc.tile_pool(name="persist", bufs=1))
        work = ctx.enter_context(tc.tile_pool(name="work", bufs=1))
        stream = ctx.enter_context(tc.tile_pool(name="stream", bufs=2))
        sloop = ctx.enter_context(tc.tile_pool(name="sloop", bufs=2))
        tail = ctx.enter_context(tc.tile_pool(name="tail", bufs=1))
        small = ctx.enter_context(tc.tile_pool(name="small", bufs=1))
        psum = ctx.enter_context(tc.tile_pool(name="psum", bufs=6, space="PSUM"))

        MM = nc.tensor.matmul
        AF = mybir.ActivationFunctionType
        OP = mybir.AluOpType
        X = mybir.AxisListType
        PAIRS = [[0, 4], [1, 5], [2, 6], [3, 7]]
        ALL8 = [list(range(8))]

        # ------- encoder (all 4 batches locally -> exact BN stats) --------
        enc_w_t = const.tile([45, HD], FP16)
        nc.sync.dma_start(enc_w_t[:], enc_w2[:])
        enc_g_t = const.tile([HD, 1], F32)
        nc.sync.dma_start(enc_g_t[:], enc_g[:])
        enc_be_t = const.tile([HD, 1], F32)
        nc.sync.dma_start(enc_be_t[:], enc_be[:])

        t_t = persist.tile([HD, 4 + L], F32)
        nc.vector.memset(t_t[:, 0:4], 0.0)
        s1p = small.tile([HD, 32], F32, tag="s1p")
        s2p = small.tile([HD, 32], F32, tag="s2p")
        for n in range(32):
            cin = stream.tile([45, CS], FP16, tag="enc_cin")
            nc.sync.dma_start(cin[:], enc_in[:, bass.ts(n, CS)])
            pt = psum.tile([HD, CS], F32, tag="mm512")
            MM(pt[:], enc_w_t[:], cin[:], start=True, stop=True)
            if n < NCHUNK:
                dst = t_t[:, 4 + n * CS : 4 + (n + 1) * CS]
            else:
                scratch = stream.tile([HD, CS], F32, tag="enc_scr")
                dst = scratch[:]
            nc.scalar.activation(dst, pt[:], AF.Copy,
                                 accum_out=s1p[:, n : n + 1])
            sq = stream.tile([HD, CS], F32, tag="enc_scr2")
            nc.scalar.activation(sq[:], pt[:], AF.Square,
                                 accum_out=s2p[:, n : n + 1])
        s1 = small.tile([HD, 1], F32, tag="s1")
        s2 = small.tile([HD, 1], F32, tag="s2")
        nc.vector.tensor_reduce(s1[:], s1p[:], axis=X.X, op=OP.add)
        nc.vector.tensor_reduce(s2[:], s2p[:], axis=X.X, op=OP.add)

        def bn_scale_bias(s1ap, s2ap, n_elems, g_ap, be_ap, tag):
            inv_n = 1.0 / n_elems
            mean = small.tile([HD, 1], F32, tag=tag + "m")
            nc.vector.tensor_scalar_mul(mean[:], s1ap, inv_n)
            m2 = small.tile([HD, 1], F32, tag=tag + "m2")
            nc.vector.tensor_tensor(m2[:], mean[:], mean[:], OP.mult)
            var = small.tile([HD, 1], F32, tag=tag + "v")
            nc.vector.scalar_tensor_tensor(var[:], s2ap, inv_n, m2[:],
                                           OP.mult, OP.subtract)
            veps = small.tile([HD, 1], F32, tag=tag + "ve")
            nc.vector.tensor_scalar_add(veps[:], var[:], 1e-5)
            rv = small.tile([HD, 1], F32, tag=tag + "rv")
            nc.vector.reciprocal(rv[:], veps[:])
            rstd = small.tile([HD, 1], F32, tag=tag + "rs")
            nc.scalar.activation(rstd[:], rv[:], AF.Sqrt)
            scale = small.tile([HD, 1], F32, tag=tag + "sc")
            nc.vector.tensor_tensor(scale[:], g_ap, rstd[:], OP.mult)
            nscale = small.tile([HD, 1], F32, tag=tag + "ns")
            nc.vector.tensor_scalar_mul(nscale[:], scale[:], -1.0)
            bias = small.tile([HD, 1], F32, tag=tag + "bi")
            nc.vector.scalar_tensor_tensor(bias[:], mean[:], nscale[:], be_ap,
                                           OP.mult, OP.add)
            return scale, bias

        sc0, bi0 = bn_scale_bias(s1[:], s2[:], B * L,
                                 enc_g_t[:], enc_be_t[:], "bn0")

        nc.scalar.activation(t_t[:, 4:], t_t[:, 4:], AF.Relu,
                             bias=bi0[:], scale=sc0[:])
        t16 = persist.tile([HD, 4 + L], FP16)
        nc.vector.memset(t16[:, 0:4], 0.0)
        nc.scalar.activation(t16[:, 4:], t_t[:, 4:], AF.Copy)

        # ---------------- trunk weights ----------------
        iptap_t = const.tile([HD, NL * DC * DI], FP16)
        nc.sync.dma_start(iptap_t[:], ip_tap[:])
        ipz_t = const.tile([HD, NL * DI], FP16)
        nc.sync.dma_start(ipz_t[:], ip_z[:])
        convb_t = const.tile([DI, NL], F32)
        nc.sync.dma_start(convb_t[:], conv_b[:])
        wd_t = const.tile([DI, NL * DI], BF16)
        nc.sync.dma_start(wd_t[:], wd_T[:])
        bct_t = const.tile([DI, NL * 2 * DSL], BF16)
        nc.sync.dma_start(bct_t[:], bc_T[:])
        dtb_t = const.tile([DI, NL], F32)
        nc.sync.dma_start(dtb_t[:], dt_b[:])
        acols_t = const.tile([DI, NL * DSL], F32)
        nc.sync.dma_start(acols_t[:], a_cols[:])
        dcol_t = const.tile([DI, NL], F32)
        nc.sync.dma_start(dcol_t[:], d_col[:])
        opt_t = const.tile([DI, NL * HD], BF16)
        nc.sync.dma_start(opt_t[:], op_T[:])

        hmid = persist.tile([DI, DSL], F32)

        # decoder tiles/helpers created up front: pad memsets run during the
        # (DVE-idle) encoder, and the last layer can start dec1 early
        d1_taps = const.tile([HD, 9 * HD], FP16)
        nc.sync.dma_start(d1_taps[:], dec1_tap[:])
        padA = persist.tile([HD, PADL], FP16)
        nc.vector.memset(padA[:], 0.0)
        padB = persist.tile([HD, PADL], FP16)
        nc.vector.memset(padB[:], 0.0)

        def interior(tile_ap):
            return tile_ap[:, PBASE : PBASE + PW * H].rearrange(
                "p (h w) -> p h w", w=PW)[:, :, 0:W]

        def conv9(dst_tile, src_tile, taps_tile, m_out, tapw, evict,
                  n0=0, n1=None):
            total = PW * H
            nch = (total + CS - 1) // CS
            for n in range(n0, nch if n1 is None else n1):
                c0 = PBASE + n * CS
                cw = min(CS, PBASE + total - c0)
                pt = psum.tile([m_out, CS], F32, tag="mm512")
                for ti in range(9):
                    dy, dx = ti // 3, ti % 3
                    off = c0 + (dy - 1) * PW + (dx - 1)
                    MM(pt[:, 0:cw],
                       taps_tile[:, ti * tapw : ti * tapw + m_out],
                       src_tile[:, off : off + cw],
                       start=(ti == 0), stop=(ti == 8))
                evict(dst_tile[0:m_out, c0 : c0 + cw], pt[:, 0:cw])

        d1_evict = lambda d, p: nc.scalar.activation(d, p, AF.Copy)

        # ---------------- trunk (software-pipelined) ----------------
        # Per layer: [1a 2a] issued by previous iteration; s0 | 1b 2b |
        # tail0(coll#1) | s1 | 1a'(next) 2a'(next) | tail1(coll#2).
        # Residual updates (DVE) are injected mid-s-phase so their semaphore
        # waits never head-block the DVE queue.
        INJ = 5   # inject deferred t-updates after this many scan states

        def mk_layer(li):
            xi_c = work.tile([DI, L], BF16, tag="xi_c")
            sz = work.tile([DI, L], BF16, tag="sz")
            dlt = work.tile([DI, L], BF16, tag="dlt")
            bc_sb = work.tile([2 * DSL, L], BF16, tag="bc_sb")
            du = work.tile([DI, L], BF16, tag="du")
            ly = {"li": li, "xi_c": xi_c, "sz": sz, "dlt": dlt,
                  "bc_sb": bc_sb, "du": du}

            def chunk1(n):
                p_xi = psum.tile([DI, CS], F32, tag="mm512")
                for k in range(DC):
                    MM(p_xi[:],
                       iptap_t[:, (li * DC + k) * DI : (li * DC + k + 1) * DI],
                       t16[:, 1 + k + n * CS : 1 + k + n * CS + CS],
                       start=(k == 0), stop=(k == DC - 1))
                nc.scalar.activation(ly["xi_c"][:, bass.ts(n, CS)], p_xi[:],
                                     AF.Silu,
                                     bias=convb_t[:, li : li + 1], scale=1.0)
                p_z = psum.tile([DI, CS], F32, tag="mm512")
                MM(p_z[:], ipz_t[:, li * DI : (li + 1) * DI],
                   t16[:, 4 + n * CS : 4 + (n + 1) * CS],
                   start=True, stop=True)
                nc.scalar.activation(ly["sz"][:, bass.ts(n, CS)], p_z[:],
                                     AF.Silu)

            def chunk2(n, ev):
                p_d = psum.tile([DI, CS], F32, tag="mm512")
                MM(p_d[:], wd_t[:, li * DI : (li + 1) * DI],
                   ly["xi_c"][:, bass.ts(n, CS)], start=True, stop=True)
                # ev = exp(p_d + dt_b); delta = ln(1 + ev) applied in s_half.
                # Exp/Ln share the natural_log_exp act table -> no reloads.
                nc.scalar.activation(ev[:, bass.ts(n % 4, CS)], p_d[:],
                                     AF.Exp,
                                     bias=dtb_t[:, li : li + 1], scale=1.0)
                p_bc = psum.tile([2 * DSL, CS], F32, tag="mm512")
                MM(p_bc[:], bct_t[:, li * 2 * DSL : (li + 1) * 2 * DSL],
                   ly["xi_c"][:, bass.ts(n, CS)], start=True, stop=True)
                nc.scalar.activation(ly["bc_sb"][:, bass.ts(n, CS)], p_bc[:],
                                     AF.Copy)

            def half_chunks(hf):
                for n in range(hf * 4, hf * 4 + 4):
                    chunk1(n)
                ev = tail.tile([DI, LH], F32, tag=f"ev{hf}")
                for n in range(hf * 4, hf * 4 + 4):
                    chunk2(n, ev)
                sl = slice(hf * LH, (hf + 1) * LH)
                nc.sync.dma_start(bc_dram[li][:, sl], ly["bc_sb"][:, sl])
                return ev

            def s_prologue(hf, ev):
                # issued ahead of the half tails so the Ln never queues
                # behind the tail's PSUM evictions on the Act engine
                sl = slice(hf * LH, (hf + 1) * LH)
                dlt, du, xi_c = ly["dlt"], ly["du"], ly["xi_c"]
                nc.scalar.activation(dlt[:, sl], ev[:], AF.Ln, bias=1.0,
                                     scale=1.0)
                nc.vector.tensor_tensor(du[:, sl], dlt[:, sl], xi_c[:, sl],
                                        OP.mult)

            def s_half(hf, inject=None):
                sl = slice(hf * LH, (hf + 1) * LH)
                dlt, du, xi_c = ly["dlt"], ly["du"], ly["xi_c"]
                acc = None
                pend = None
                for s in range(DSL):
                    if s == INJ and inject is not None:
                        inject()
                    dA = sloop.tile([DI, LH], FP16, tag="dA")
                    nc.scalar.activation(
                        dA[:], dlt[:, sl], AF.Exp,
                        scale=acols_t[:, li * DSL + s : li * DSL + s + 1])
                    brep = sloop.tile([DI, LH], BF16, tag="brep")
                    nc.sync.dma_start(
                        brep[:],
                        bc_dram[li][s : s + 1, sl].broadcast_to((DI, LH)))
                    crep = sloop.tile([DI, LH], BF16, tag="crep")
                    nc.sync.dma_start(
                        crep[:],
                        bc_dram[li][DSL + s : DSL + s + 1, sl].broadcast_to(
                            (DI, LH)))
                    xs = sloop.tile([DI, LH], BF16, tag="xs")
                    nc.vector.tensor_tensor(xs[:], du[:, sl], brep[:], OP.mult)
                    hs = sloop.tile([DI, LH], BF16, tag="hs")
                    init = 0.0 if hf == 0 else hmid[:, s : s + 1]
                    nc.vector.tensor_tensor_scan(hs[:], dA[:], xs[:], init,
                                                 OP.mult, OP.add)
                    if hf == 0:
                        nc.vector.tensor_copy(hmid[:, s : s + 1],
                                              hs[:, LH - 1 : LH])
                    if pend is not None:
                        anew = sloop.tile([DI, LH], BF16, tag=f"acc{hf}")
                        if acc is None:
                            # fold the D*u term into the first accumulate
                            nc.vector.scalar_tensor_tensor(
                                anew[:], xi_c[:, sl],
                                dcol_t[:, li : li + 1], pend[:],
                                OP.mult, OP.add)
                        else:
                            nc.vector.tensor_tensor(anew[:], acc[:], pend[:],
                                                    OP.add)
                        acc = anew
                    hc = sloop.tile([DI, LH], BF16, tag="hc")
                    nc.vector.tensor_tensor(hc[:], hs[:], crep[:], OP.mult)
                    pend = hc
                anew = sloop.tile([DI, LH], BF16, tag=f"acc{hf}")
                nc.vector.tensor_tensor(anew[:], acc[:], pend[:], OP.add)
                return anew

            def half_tail(hf, acc):
                sl = slice(hf * LH, (hf + 1) * LH)
                yg = tail.tile([DI, LH], BF16, tag=f"yg{hf}")
                nc.vector.tensor_tensor(yg[:], acc[:], ly["sz"][:, sl],
                                        OP.mult)
                dt_part = tail.tile([HD, LH], BF16, tag=f"dtp{hf}")
                for n in range(4):
                    p_o = psum.tile([HD, CS], F32, tag="mm512")
                    MM(p_o[:], opt_t[:, li * HD : (li + 1) * HD],
                       yg[:, bass.ts(n, CS)], start=True, stop=True)
                    nc.scalar.activation(dt_part[:, bass.ts(n, CS)], p_o[:],
                                         AF.Copy)
                nc.sync.dma_start(y_in[hf][:], dt_part[:])
                nc.gpsimd.collective_compute(
                    "AllReduce", OP.add, replica_groups=PAIRS,
                    ins=[y_in[hf][:]], outs=[y_out[hf][:]])

            def half_update(hf):
                sl = slice(4 + hf * LH, 4 + (hf + 1) * LH)
                # reuse dtp slot: its DMA into y_in completes before the
                # collective can produce y_out
                dtsum = tail.tile([HD, LH], BF16, tag=f"dtp{hf}")
                nc.sync.dma_start(dtsum[:], y_out[hf][:])
                nc.vector.tensor_tensor(t_t[:, sl], t_t[:, sl], dtsum[:],
                                        OP.add)
                if li < NL - 1:
                    nc.vector.tensor_copy(t16[:, sl], t_t[:, sl])

            ly.update(chunk1=chunk1, chunk2=chunk2, half_chunks=half_chunks,
                      s_prologue=s_prologue, s_half=s_half,
                      half_tail=half_tail, half_update=half_update)
            return ly

        cur = mk_layer(0)
        ev0 = cur["half_chunks"](0)
        cur["s_prologue"](0, ev0)
        pend_update = None
        for li in range(NL):
            acc0 = cur["s_half"](0, inject=pend_update)
            ev1 = cur["half_chunks"](1)
            cur["s_prologue"](1, ev1)
            cur["half_tail"](0, acc0)
            hu = cur["half_update"]
            acc1 = cur["s_half"](1, inject=lambda hu=hu: hu(0))
            if li + 1 < NL:
                nxt = mk_layer(li + 1)
                ev0 = nxt["half_chunks"](0)
                nxt["s_prologue"](0, ev0)
            else:
                # early decoder: top-half dec1 runs under the last collectives
                nc.scalar.activation(
                    interior(padA)[:, 0:32, :],
                    t_t[:, 4 : 4 + LH].rearrange("p (h w) -> p h w", w=W),
                    AF.Copy)
                conv9(padB, padA, d1_taps, HD, HD, d1_evict, n0=0, n1=3)
            cur["half_tail"](1, acc1)
            pend_update = lambda hu=hu: hu(1)
            if li + 1 < NL:
                cur = nxt

        # ---------------- decoder ----------------
        d2_taps = const.tile([HD, 9 * NFH], FP16)
        nc.sync.dma_start(d2_taps[:], dec2_tap[:])
        d1g_t = const.tile([HD, 1], F32)
        nc.sync.dma_start(d1g_t[:], dec1_g[:])
        d1be_t = const.tile([HD, 1], F32)
        nc.sync.dma_start(d1be_t[:], dec1_be[:])
        d2b_t = const.tile([NFH, 1], F32)
        nc.sync.dma_start(d2b_t[:], dec2_b[:])
        out_pad = persist.tile([NFH, PADL], BF16)

        pend_update()       # last layer's half-1 residual into t_t
        nc.scalar.activation(
            interior(padA)[:, 32:64, :],
            t_t[:, 4 + LH :].rearrange("p (h w) -> p h w", w=W),
            AF.Copy)
        conv9(padB, padA, d1_taps, HD, HD, d1_evict, n0=3)

        d1_int = interior(padB)
        ds1 = small.tile([HD, 1], F32, tag="ds1")
        nc.vector.tensor_reduce(ds1[:], d1_int, axis=X.XY, op=OP.add)
        ds2 = small.tile([HD, 1], F32, tag="ds2")
        nc.scalar.activation(interior(padA), d1_int, AF.Square,
                             accum_out=ds2[:])
        packed = small.tile([HD, 2], F32, tag="pk")
        nc.vector.tensor_copy(packed[:, 0:1], ds1[:])
        nc.vector.tensor_copy(packed[:, 1:2], ds2[:])
        nc.sync.dma_start(cc_in[:], packed[:])
        # 4-way: each group holds all 4 batches exactly once -> exact stats
        nc.gpsimd.collective_compute(
            "AllReduce", OP.add,
            replica_groups=[[0, 1, 2, 3], [4, 5, 6, 7]],
            ins=[cc_in[:]], outs=[cc_out[:]])
        red = small.tile([HD, 2], F32, tag="red")
        nc.sync.dma_start(red[:], cc_out[:])
        sc1, bi1 = bn_scale_bias(red[:, 0:1], red[:, 1:2], B * L,
                                 d1g_t[:], d1be_t[:], "bn1")

        # h2 into padA (pads still zero outside interior; square scratch
        # gets overwritten)
        nc.scalar.activation(interior(padA), d1_int, AF.Relu,
                             bias=bi1[:], scale=sc1[:])
        conv9(out_pad, padA, d2_taps, NFH, NFH,
              lambda d, p: nc.scalar.activation(
                  d, p, AF.Identity, bias=d2b_t[:], scale=1.0))
        out_int = out_pad[:NFH, PBASE : PBASE + PW * H].rearrange(
            "p (h w) -> p h w", w=PW)[:, :, 0:W]
        nc.sync.dma_start(out_ext[:].rearrange("p (h w) -> p h w", w=W),
                          out_int)

    split_excess_waits(nc)
    return nc


_CACHED = {}


def _get_kernel():
    if "nc" not in _CACHED:
        _CACHED["nc"] = build_kernel()
    return _CACHED["nc"]


def _host_inputs(inputs):
    f32 = np.float32
    bf16 = ml_dtypes.bfloat16
    x = np.asarray(inputs["x"], f32)
    enc_w = np.asarray(inputs["enc_w"], f32)
    in_proj = np.asarray(inputs["in_proj"], f32)
    conv_w = np.asarray(inputs["conv_w"], f32)
    x_proj = np.asarray(inputs["x_proj"], f32)
    dt_w = np.asarray(inputs["dt_w"], f32)
    A_log = np.asarray(inputs["A_log"], f32)
    out_proj = np.asarray(inputs["out_proj"], f32)
    dec1_w = np.asarray(inputs["dec1_w"], f32)
    dec2_w = np.asarray(inputs["dec2_w"], f32)

    xp = np.zeros((B, NB, H + 2, W + 2), f32)
    xp[:, :, 1:-1, 1:-1] = x
    cols = np.empty((NB, 3, 3, B, L), f32)
    for dy in range(3):
        for dx in range(3):
            cols[:, dy, dx] = (
                xp[:, :, dy : dy + H, dx : dx + W]
                .reshape(B, NB, L).transpose(1, 0, 2))
    cols_b = cols.reshape(45, B, L)
    enc_w2 = np.ascontiguousarray(enc_w.reshape(HD, 45).T)

    ip_tap = np.empty((HD, NL, DC, DI), f32)
    ip_z = np.empty((HD, NL, DI), f32)
    wd_T = np.empty((DI, NL, DI), f32)
    bc_full = np.empty((DI, NL, 2 * DS), f32)
    a_full = np.empty((DI, NL, DS), f32)
    op_T = np.empty((DI, NL, HD), f32)
    for i in range(NL):
        for k in range(DC):
            ip_tap[:, i, k, :] = (conv_w[i][:, k : k + 1] * in_proj[i][:DI]).T
        ip_z[:, i, :] = in_proj[i][DI:].T
        wd_T[:, i, :] = (dt_w[i] @ x_proj[i][:DTR]).T
        bc_full[:, i, :] = x_proj[i][DTR:].T
        a_full[:, i, :] = -np.exp(A_log[i])   # dA = exp(a_col * delta)
        op_T[:, i, :] = out_proj[i].T

    dec1_tap = np.empty((HD, 9, HD), f32)
    dec2_tap = np.empty((HD, 9, NF), f32)
    for ti in range(9):
        dy, dx = ti // 3, ti % 3
        dec1_tap[:, ti, :] = dec1_w[:, :, dy, dx].T
        dec2_tap[:, ti, :] = dec2_w[:, :, dy, dx].T
    dec2_b_full = np.asarray(inputs["dec2_b"], f32)

    common = {
        "enc_w2": enc_w2.astype(np.float16),
        "enc_g": np.asarray(inputs["enc_g"], f32).reshape(HD, 1),
        "enc_be": np.asarray(inputs["enc_be"], f32).reshape(HD, 1),
        "ip_tap": ip_tap.reshape(HD, NL * DC * DI).astype(np.float16),
        "ip_z": ip_z.reshape(HD, NL * DI).astype(np.float16),
        "conv_b": np.ascontiguousarray(
            np.asarray(inputs["conv_b"], f32).T),           # (DI, NL)
        "wd_T": wd_T.reshape(DI, NL * DI).astype(bf16),

        "dt_b": np.ascontiguousarray(np.asarray(inputs["dt_b"], f32).T),

        "d_col": np.ascontiguousarray(np.asarray(inputs["Dp"], f32).T) / 2.0,
        "op_T": op_T.reshape(DI, NL * HD).astype(bf16),
        "dec1_tap": dec1_tap.reshape(HD, 9 * HD).astype(np.float16),
        "dec1_g": np.asarray(inputs["dec1_g"], f32).reshape(HD, 1),
        "dec1_be": np.asarray(inputs["dec1_be"], f32).reshape(HD, 1),
    }
    in_maps = []
    for c in range(8):
        b0 = c % B
        sr = (c // B) * DSL
        ch0 = (c // B) * NFH
        m = dict(common)
        order = [b0] + [bb for bb in range(B) if bb != b0]
        m["enc_im2col"] = np.ascontiguousarray(
            cols_b[:, order, :].reshape(45, B * L)).astype(np.float16)
        bcs = np.concatenate(
            [bc_full[:, :, sr : sr + DSL],
             bc_full[:, :, DS + sr : DS + sr + DSL]], axis=2)
        m["bc_T"] = np.ascontiguousarray(
            bcs.reshape(DI, NL * 2 * DSL)).astype(bf16)
        m["a_cols"] = np.ascontiguousarray(
            a_full[:, :, sr : sr + DSL].reshape(DI, NL * DSL))
        m["dec2_tap"] = np.ascontiguousarray(
            dec2_tap[:, :, ch0 : ch0 + NFH].reshape(HD, 9 * NFH)
        ).astype(np.float16)
        m["dec2_b"] = np.ascontiguousarray(
            dec2_b_full[ch0 : ch0 + NFH].reshape(NFH, 1))
        in_maps.append(m)
    return in_maps


def kernel(**inputs):
    nc = _get_kernel()
    in_maps = _host_inputs(inputs)
    res = run_bass_kernel_spmd(nc, in_maps, core_ids=list(range(8)))
    out = np.empty((B, NF, H, W), np.float32)
    for b_ in range(B):
        out[b_, :NFH] = np.asarray(
            res.results[b_]["out"], np.float32).reshape(NFH, H, W)
        out[b_, NFH:] = np.asarray(
            res.results[b_ + B]["out"], np.float32).reshape(NFH, H, W)
    return out


if __name__ == "__main__":
    sys.path.insert(0, "/root/problem")
    import reference as ref

    inp = {k: np.asarray(v) for k, v in ref.setup_inputs().items()}
    got = kernel(**inp)
    print("kernel ran, output shape:", got.shape)


# revision 52
# speedup vs baseline: 1.0161x; 1.0161x over previous
"""Trainium2 Bass kernel for MinimalEventMamba.

kernel(**inputs) takes FULL inputs (as from setup_inputs()) and returns the
FULL (4, 10, 64, 64) float32 output. Internally: batch-parallel across 8
NeuronCores (4 batches x2 replicated, state-split across the pair), one SPMD
Bass kernel launch, host assembles the output.

Per-core layout: channel-on-partition, L=4096 on free dim.
- encoder conv as im2col matmul (own batch only; BN stats via 8-way AllReduce)
- mamba trunk: depthwise conv folded into in_proj taps (shifted matmuls,
  PSUM accumulate), dt_w folded into x_proj, Silu/Softplus computed as single
  fused Act ops on PSUM eviction, selective scan via tensor_tensor_scan on
  DVE per state dim, B/C rows broadcast across partitions via DRAM-bounce
  DMA, hc-multiplies offloaded to GpSimd, residual updates on GpSimd.
- per-layer y AllReduce split into two halves pipelined behind the scan
  phase (half-0 collective runs under half-1 scans; half-1 collective runs
  under the next layer's chunk work).
- decoder: dec1 duplicated per pair (cross-batch BN stats via AllReduce),
  dec2 split by output channel across the pair (5+5).
"""
import sys
import types

sys.path.insert(0, "/opt/trn_rl_repo")
sys.path.insert(0, "/opt/trn_rl_repo/concourse")
try:
    from antenv import axon_hooks  # noqa: F401
except ImportError:
    try:
        from trn_agent_boot.trn_boot import _ntff_profile_via_ctypes
        _m = types.ModuleType("antenv.axon_hooks")
        _h = _ntff_profile_via_ctypes("/opt/axon/libaxon_pjrt.so")
        _m.get_axon_ntff_profile_hook = lambda: _h
        _m.set_axon_ntff_profile_hook = lambda h: None
        sys.modules["antenv.axon_hooks"] = _m
    except Exception:
        pass

from contextlib import ExitStack

import numpy as np
import ml_dtypes

import concourse.bass as bass
import concourse.tile as tile
from concourse import mybir
from concourse.bass_utils import run_bass_kernel_spmd
import bass_rust

F32 = mybir.dt.float32
F32R = mybir.dt.float32r
BF16 = mybir.dt.bfloat16
FP16 = mybir.dt.float16

NB, HD, NL, NF = 5, 64, 4, 10
DI, DS, DC, DTR = 128, 16, 4, 4
B, H, W = 4, 64, 64
L = H * W                     # 4096
PW = W + 2                    # padded row stride 66
PADL = PW * (H + 2) + 4       # padded spatial + guard cols (4360)
PBASE = 1 + PW + 1            # first interior col in padded layout
NCHUNK = 8                    # L / 512
CS = 512
LH = L // 2                   # half length (2048)
DSL = DS // 2                 # states per core (s-split across core pairs)
NFH = NF // 2                 # dec2 output channels per core

HC_ON_POOL = False            # GpSimd elementwise measured 4x slower than DVE
                              # 2x mode AND contends for SBUF ports with DVE


def split_excess_waits(nc, max_waits=1):
    """This container's walrus accepts only 1 sync wait per instruction;
    move overflow waits onto NOPs inserted before the offending op."""
    f = nc.m.functions[0]
    for bb in f.blocks:
        insts = bb.instructions
        i = 0
        while i < len(insts):
            inst = insts[i]
            si = inst.sync_info
            if si is not None and len(si.on_wait) > max_waits:
                waits = list(si.on_wait)
                si.on_wait = waits[-max_waits:]
                inst.sync_info = si
                overflow = waits[:-max_waits]
                eng = nc.engines[inst.engine]
                pos = i
                for j in range(0, len(overflow), max_waits):
                    nop = eng.nop(hint="splitw", nofuse=True)
                    nop_inst = nop.ins if hasattr(nop, "ins") else nop
                    for bb2 in f.blocks:
                        if any(x is nop_inst for x in bb2.instructions):
                            bb2.instructions[:] = [
                                x for x in bb2.instructions if x is not nop_inst
                            ]
                            break
                    nop_inst.sync_info = bass_rust.SyncInfo(
                        on_wait=overflow[j : j + max_waits], on_update=[]
                    )
                    insts.insert(pos, nop_inst)
                    pos += 1
                i = pos + 1
            else:
                i += 1


def build_kernel():
    nc = bass.Bass()
    dp = nc.declare_dram_parameter

    enc_in = dp("enc_im2col", [45, B * L], FP16, isOutput=False)
    enc_w2 = dp("enc_w2", [45, HD], FP16, isOutput=False)
    enc_g = dp("enc_g", [HD, 1], F32, isOutput=False)
    enc_be = dp("enc_be", [HD, 1], F32, isOutput=False)
    ip_tap = dp("ip_tap", [HD, NL * DC * DI], FP16, isOutput=False)
    ip_z = dp("ip_z", [HD, NL * DI], FP16, isOutput=False)
    conv_b = dp("conv_b", [DI, NL], F32, isOutput=False)
    wd_T = dp("wd_T", [DI, NL * DI], BF16, isOutput=False)
    bc_T = dp("bc_T", [DI, NL * 2 * DSL], BF16, isOutput=False)
    dt_b = dp("dt_b", [DI, NL], F32, isOutput=False)
    a_cols = dp("a_cols", [DI, NL * DSL], F32, isOutput=False)
    d_col = dp("d_col", [DI, NL], F32, isOutput=False)
    op_T = dp("op_T", [DI, NL * HD], BF16, isOutput=False)
    dec1_tap = dp("dec1_tap", [HD, 9 * HD], FP16, isOutput=False)
    dec1_g = dp("dec1_g", [HD, 1], F32, isOutput=False)
    dec1_be = dp("dec1_be", [HD, 1], F32, isOutput=False)
    dec2_tap = dp("dec2_tap", [HD, 9 * NFH], FP16, isOutput=False)
    dec2_b = dp("dec2_b", [NFH, 1], F32, isOutput=False)

    out_ext = dp("out", [NFH, L], BF16, isOutput=True)

    bc_dram = nc.dram_tensor("bc_dram", [NL, 2 * DSL, L], BF16)
    y_in = [nc.dram_tensor(f"y_in{h}", [HD, LH], BF16) for h in range(2)]
    y_out = [nc.dram_tensor(f"y_out{h}", [HD, LH], BF16) for h in range(2)]
    cc_in = nc.dram_tensor("cc_in", [HD, 2], F32)
    cc_out = nc.dram_tensor("cc_out", [HD, 2], F32)

    ctx = ExitStack()
    with ctx:
        tc = ctx.enter_context(tile.TileContext(nc))
        const = ctx.enter_context(tc.tile_pool(name="const", bufs=1))
        persist = ctx.enter_context(tc.tile_pool(name="persist", bufs=1))
        work = ctx.enter_context(tc.tile_pool(name="work", bufs=1))
        stream = ctx.enter_context(tc.tile_pool(name="stream", bufs=2))
        sloop = ctx.enter_context(tc.tile_pool(name="sloop", bufs=2))
        tail = ctx.enter_context(tc.tile_pool(name="tail", bufs=1))
        small = ctx.enter_context(tc.tile_pool(name="small", bufs=1))
        psum = ctx.enter_context(tc.tile_pool(name="psum", bufs=6, space="PSUM"))

        MM = nc.tensor.matmul
        AF = mybir.ActivationFunctionType
        OP = mybir.AluOpType
        X = mybir.AxisListType
        PAIRS = [[0, 4], [1, 5], [2, 6], [3, 7]]
        ALL8 = [list(range(8))]

        # ------- encoder (all 4 batches locally -> exact BN stats) --------
        enc_w_t = const.tile([45, HD], FP16)
        nc.sync.dma_start(enc_w_t[:], enc_w2[:])
        enc_g_t = const.tile([HD, 1], F32)
        nc.sync.dma_start(enc_g_t[:], enc_g[:])
        enc_be_t = const.tile([HD, 1], F32)
        nc.sync.dma_start(enc_be_t[:], enc_be[:])

        t_t = persist.tile([HD, 4 + L], F32)
        nc.vector.memset(t_t[:, 0:4], 0.0)
        s1p = small.tile([HD, 32], F32, tag="s1p")
        s2p = small.tile([HD, 32], F32, tag="s2p")
        for n in range(32):
            cin = stream.tile([45, CS], FP16, tag="enc_cin")
            nc.sync.dma_start(cin[:], enc_in[:, bass.ts(n, CS)])
            pt = psum.tile([HD, CS], F32, tag="mm512")
            MM(pt[:], enc_w_t[:], cin[:], start=True, stop=True)
            if n < NCHUNK:
                dst = t_t[:, 4 + n * CS : 4 + (n + 1) * CS]
            else:
                scratch = stream.tile([HD, CS], F32, tag="enc_scr")
                dst = scratch[:]
            nc.scalar.activation(dst, pt[:], AF.Copy,
                                 accum_out=s1p[:, n : n + 1])
            sq = stream.tile([HD, CS], F32, tag="enc_scr2")
            nc.scalar.activation(sq[:], pt[:], AF.Square,
                                 accum_out=s2p[:, n : n + 1])
        s1 = small.tile([HD, 1], F32, tag="s1")
        s2 = small.tile([HD, 1], F32, tag="s2")
        nc.vector.tensor_reduce(s1[:], s1p[:], axis=X.X, op=OP.add)
        nc.vector.tensor_reduce(s2[:], s2p[:], axis=X.X, op=OP.add)

        def bn_scale_bias(s1ap, s2ap, n_elems, g_ap, be_ap, tag):
            inv_n = 1.0 / n_elems
            mean = small.tile([HD, 1], F32, tag=tag + "m")
            nc.vector.tensor_scalar_mul(mean[:], s1ap, inv_n)
            m2 = small.tile([HD, 1], F32, tag=tag + "m2")
            nc.vector.tensor_tensor(m2[:], mean[:], mean[:], OP.mult)
            var = small.tile([HD, 1], F32, tag=tag + "v")
            nc.vector.scalar_tensor_tensor(var[:], s2ap, inv_n, m2[:],
                                           OP.mult, OP.subtract)
            veps = small.tile([HD, 1], F32, tag=tag + "ve")
            nc.vector.tensor_scalar_add(veps[:], var[:], 1e-5)
            rv = small.tile([HD, 1], F32, tag=tag + "rv")
            nc.vector.reciprocal(rv[:], veps[:])
            rstd = small.tile([HD, 1], F32, tag=tag + "rs")
            nc.scalar.activation(rstd[:], rv[:], AF.Sqrt)
            scale = small.tile([HD, 1], F32, tag=tag + "sc")
            nc.vector.tensor_tensor(scale[:], g_ap, rstd[:], OP.mult)
            nscale = small.tile([HD, 1], F32, tag=tag + "ns")
            nc.vector.tensor_scalar_mul(nscale[:], scale[:], -1.0)
            bias = small.tile([HD, 1], F32, tag=tag + "bi")
            nc.vector.scalar_tensor_tensor(bias[:], mean[:], nscale[:], be_ap,
                                           OP.mult, OP.add)
            return scale, bias

        sc0, bi0 = bn_scale_bias(s1[:], s2[:], B * L,
                                 enc_g_t[:], enc_be_t[:], "bn0")

        nc.scalar.activation(t_t[:, 4:], t_t[:, 4:], AF.Relu,
                             bias=bi0[:], scale=sc0[:])
        t16 = persist.tile([HD, 4 + L], FP16)
        nc.vector.memset(t16[:, 0:4], 0.0)
        nc.scalar.activation(t16[:, 4:], t_t[:, 4:], AF.Copy)

        # ---------------- trunk weights ----------------
        iptap_t = const.tile([HD, NL * DC * DI], FP16)
        nc.sync.dma_start(iptap_t[:], ip_tap[:])
        ipz_t = const.tile([HD, NL * DI], FP16)
        nc.sync.dma_start(ipz_t[:], ip_z[:])
        convb_t = const.tile([DI, NL], F32)
        nc.sync.dma_start(convb_t[:], conv_b[:])
        wd_t = const.tile([DI, NL * DI], BF16)
        nc.sync.dma_start(wd_t[:], wd_T[:])
        bct_t = const.tile([DI, NL * 2 * DSL], BF16)
        nc.sync.dma_start(bct_t[:], bc_T[:])
        dtb_t = const.tile([DI, NL], F32)
        nc.sync.dma_start(dtb_t[:], dt_b[:])
        acols_t = const.tile([DI, NL * DSL], F32)
        nc.sync.dma_start(acols_t[:], a_cols[:])
        dcol_t = const.tile([DI, NL], F32)
        nc.sync.dma_start(dcol_t[:], d_col[:])
        opt_t = const.tile([DI, NL * HD], BF16)
        nc.sync.dma_start(opt_t[:], op_T[:])

        hmid = persist.tile([DI, DSL], F32)

        # decoder tiles/helpers created up front: pad memsets run during the
        # (DVE-idle) encoder, and the last layer can start dec1 early
        d1_taps = const.tile([HD, 9 * HD], FP16)
        nc.sync.dma_start(d1_taps[:], dec1_tap[:])
        padA = persist.tile([HD, PADL], FP16)
        nc.vector.memset(padA[:], 0.0)
        padB = persist.tile([HD, PADL], FP16)
        nc.vector.memset(padB[:], 0.0)

        def interior(tile_ap):
            return tile_ap[:, PBASE : PBASE + PW * H].rearrange(
                "p (h w) -> p h w", w=PW)[:, :, 0:W]

        def conv9(dst_tile, src_tile, taps_tile, m_out, tapw, evict,
                  n0=0, n1=None):
            total = PW * H
            nch = (total + CS - 1) // CS
            for n in range(n0, nch if n1 is None else n1):
                c0 = PBASE + n * CS
                cw = min(CS, PBASE + total - c0)
                pt = psum.tile([m_out, CS], F32, tag="mm512")
                for ti in range(9):
                    dy, dx = ti // 3, ti % 3
                    off = c0 + (dy - 1) * PW + (dx - 1)
                    MM(pt[:, 0:cw],
                       taps_tile[:, ti * tapw : ti * tapw + m_out],
                       src_tile[:, off : off + cw],
                       start=(ti == 0), stop=(ti == 8))
                evict(dst_tile[0:m_out, c0 : c0 + cw], pt[:, 0:cw])

        d1_evict = lambda d, p: nc.scalar.activation(d, p, AF.Copy)

        # ---------------- trunk (software-pipelined) ----------------
        # Per layer: [1a 2a] issued by previous iteration; s0 | 1b 2b |
        # tail0(coll#1) | s1 | 1a'(next) 2a'(next) | tail1(coll#2).
        # Residual updates (DVE) are injected mid-s-phase so their semaphore
        # waits never head-block the DVE queue.
        INJ = 5   # inject deferred t-updates after this many scan states

        def mk_layer(li):
            xi_c = work.tile([DI, L], BF16, tag="xi_c")
            sz = work.tile([DI, L], BF16, tag="sz")
            dlt = work.tile([DI, L], BF16, tag="dlt")
            bc_sb = work.tile([2 * DSL, L], BF16, tag="bc_sb")
            du = work.tile([DI, L], BF16, tag="du")
            ly = {"li": li, "xi_c": xi_c, "sz": sz, "dlt": dlt,
                  "bc_sb": bc_sb, "du": du}

            def chunk1(n):
                p_xi = psum.tile([DI, CS], F32, tag="mm512")
                for k in range(DC):
                    MM(p_xi[:],
                       iptap_t[:, (li * DC + k) * DI : (li * DC + k + 1) * DI],
                       t16[:, 1 + k + n * CS : 1 + k + n * CS + CS],
                       start=(k == 0), stop=(k == DC - 1))
                nc.scalar.activation(ly["xi_c"][:, bass.ts(n, CS)], p_xi[:],
                                     AF.Silu,
                                     bias=convb_t[:, li : li + 1], scale=1.0)
                p_z = psum.tile([DI, CS], F32, tag="mm512")
                MM(p_z[:], ipz_t[:, li * DI : (li + 1) * DI],
                   t16[:, 4 + n * CS : 4 + (n + 1) * CS],
                   start=True, stop=True)
                nc.scalar.activation(ly["sz"][:, bass.ts(n, CS)], p_z[:],
                                     AF.Silu)

            def chunk2(n, ev):
                p_d = psum.tile([DI, CS], F32, tag="mm512")
                MM(p_d[:], wd_t[:, li * DI : (li + 1) * DI],
                   ly["xi_c"][:, bass.ts(n, CS)], start=True, stop=True)
                # ev = exp(p_d + dt_b); delta = ln(1 + ev) applied in s_half.
                # Exp/Ln share the natural_log_exp act table -> no reloads.
                nc.scalar.activation(ev[:, bass.ts(n % 4, CS)], p_d[:],
                                     AF.Exp,
                                     bias=dtb_t[:, li : li + 1], scale=1.0)
                p_bc = psum.tile([2 * DSL, CS], F32, tag="mm512")
                MM(p_bc[:], bct_t[:, li * 2 * DSL : (li + 1) * 2 * DSL],
                   ly["xi_c"][:, bass.ts(n, CS)], start=True, stop=True)
                nc.scalar.activation(ly["bc_sb"][:, bass.ts(n, CS)], p_bc[:],
                                     AF.Copy)

            def half_chunks(hf):
                for n in range(hf * 4, hf * 4 + 4):
                    chunk1(n)
                ev = tail.tile([DI, LH], F32, tag=f"ev{hf}")
                for n in range(hf * 4, hf * 4 + 4):
                    chunk2(n, ev)
                sl = slice(hf * LH, (hf + 1) * LH)
                nc.sync.dma_start(bc_dram[li][:, sl], ly["bc_sb"][:, sl])
                return ev

            def s_prologue(hf, ev):
                # issued ahead of the half tails so the Ln never queues
                # behind the tail's PSUM evictions on the Act engine
                sl = slice(hf * LH, (hf + 1) * LH)
                dlt, du, xi_c = ly["dlt"], ly["du"], ly["xi_c"]
                nc.scalar.activation(dlt[:, sl], ev[:], AF.Ln, bias=1.0,
                                     scale=1.0)
                nc.vector.tensor_tensor(du[:, sl], dlt[:, sl], xi_c[:, sl],
                                        OP.mult)

            def s_half(hf, inject=None):
                sl = slice(hf * LH, (hf + 1) * LH)
                dlt, du, xi_c = ly["dlt"], ly["du"], ly["xi_c"]
                acc = None
                pend = None
                for s in range(DSL):
                    if s == INJ and inject is not None:
                        inject()
                    dA = sloop.tile([DI, LH], FP16, tag="dA")
                    nc.scalar.activation(
                        dA[:], dlt[:, sl], AF.Exp,
                        scale=acols_t[:, li * DSL + s : li * DSL + s + 1])
                    brep = sloop.tile([DI, LH], BF16, tag="brep")
                    nc.sync.dma_start(
                        brep[:],
                        bc_dram[li][s : s + 1, sl].broadcast_to((DI, LH)))
                    crep = sloop.tile([DI, LH], BF16, tag="crep")
                    nc.sync.dma_start(
                        crep[:],
                        bc_dram[li][DSL + s : DSL + s + 1, sl].broadcast_to(
                            (DI, LH)))
                    xs = sloop.tile([DI, LH], BF16, tag="xs")
                    nc.vector.tensor_tensor(xs[:], du[:, sl], brep[:], OP.mult)
                    hs = sloop.tile([DI, LH], BF16, tag="hs")
                    init = 0.0 if hf == 0 else hmid[:, s : s + 1]
                    nc.vector.tensor_tensor_scan(hs[:], dA[:], xs[:], init,
                                                 OP.mult, OP.add)
                    if hf == 0:
                        nc.vector.tensor_copy(hmid[:, s : s + 1],
                                              hs[:, LH - 1 : LH])
                    if pend is not None:
                        anew = sloop.tile([DI, LH], BF16, tag=f"acc{hf}")
                        if acc is None:
                            # fold the D*u term into the first accumulate
                            nc.vector.scalar_tensor_tensor(
                                anew[:], xi_c[:, sl],
                                dcol_t[:, li : li + 1], pend[:],
                                OP.mult, OP.add)
                        else:
                            nc.vector.tensor_tensor(anew[:], acc[:], pend[:],
                                                    OP.add)
                        acc = anew
                    hc = sloop.tile([DI, LH], BF16, tag="hc")
                    nc.vector.tensor_tensor(hc[:], hs[:], crep[:], OP.mult)
                    pend = hc
                anew = sloop.tile([DI, LH], BF16, tag=f"acc{hf}")
                nc.vector.tensor_tensor(anew[:], acc[:], pend[:], OP.add)
                return anew

            def half_tail(hf, acc):
                sl = slice(hf * LH, (hf + 1) * LH)
                yg = tail.tile([DI, LH], BF16, tag=f"yg{hf}")
                nc.vector.tensor_tensor(yg[:], acc[:], ly["sz"][:, sl],
                                        OP.mult)
                dt_part = tail.tile([HD, LH], BF16, tag=f"dtp{hf}")
                for n in range(4):
                    p_o = psum.tile([HD, CS], F32, tag="mm512")
                    MM(p_o[:], opt_t[:, li * HD : (li + 1) * HD],
                       yg[:, bass.ts(n, CS)], start=True, stop=True)
                    nc.scalar.activation(dt_part[:, bass.ts(n, CS)], p_o[:],
                                         AF.Copy)
                nc.sync.dma_start(y_in[hf][:], dt_part[:])
                nc.gpsimd.collective_compute(
                    "AllReduce", OP.add, replica_groups=PAIRS,
                    ins=[y_in[hf][:]], outs=[y_out[hf][:]])

            def half_update(hf):
                sl = slice(4 + hf * LH, 4 + (hf + 1) * LH)
                # reuse dtp slot: its DMA into y_in completes before the
                # collective can produce y_out
                dtsum = tail.tile([HD, LH], BF16, tag=f"dtp{hf}")
                nc.sync.dma_start(dtsum[:], y_out[hf][:])
                nc.vector.tensor_tensor(t_t[:, sl], t_t[:, sl], dtsum[:],
                                        OP.add)
                if li < NL - 1:
                    nc.vector.tensor_copy(t16[:, sl], t_t[:, sl])

            ly.update(chunk1=chunk1, chunk2=chunk2, half_chunks=half_chunks,
                      s_prologue=s_prologue, s_half=s_half,
                      half_tail=half_tail, half_update=half_update)
            return ly

        cur = mk_layer(0)
        ev0 = cur["half_chunks"](0)
        cur["s_prologue"](0, ev0)
        pend_update = None
        for li in range(NL):
            acc0 = cur["s_half"](0, inject=pend_update)
            ev1 = cur["half_chunks"](1)
            cur["s_prologue"](1, ev1)
            cur["half_tail"](0, acc0)
            hu = cur["half_update"]
            acc1 = cur["s_half"](1, inject=lambda hu=hu: hu(0))
            if li + 1 < NL:
                nxt = mk_layer(li + 1)
                ev0 = nxt["half_chunks"](0)
                nxt["s_prologue"](0, ev0)
            cur["half_tail"](1, acc1)
            pend_update = lambda hu=hu: hu(1)
            if li + 1 < NL:
                cur = nxt

        # ---------------- decoder ----------------
        d2_taps = const.tile([HD, 9 * NFH], FP16)
        nc.sync.dma_start(d2_taps[:], dec2_tap[:])
        d1g_t = const.tile([HD, 1], F32)
        nc.sync.dma_start(d1g_t[:], dec1_g[:])
        d1be_t = const.tile([HD, 1], F32)
        nc.sync.dma_start(d1be_t[:], dec1_be[:])
        d2b_t = const.tile([NFH, 1], F32)
        nc.sync.dma_start(d2b_t[:], dec2_b[:])
        out_pad = persist.tile([NFH, PADL], BF16)

        # top half of the image is final before the last half-1 residual
        # update; start dec1 on it while the last y-collective completes
        nc.scalar.activation(
            interior(padA)[:, 0:32, :],
            t_t[:, 4 : 4 + LH].rearrange("p (h w) -> p h w", w=W),
            AF.Copy)
        conv9(padB, padA, d1_taps, HD, HD, d1_evict, n0=0, n1=3)
        pend_update()       # last layer's half-1 residual into t_t
        nc.scalar.activation(
            interior(padA)[:, 32:64, :],
            t_t[:, 4 + LH :].rearrange("p (h w) -> p h w", w=W),
            AF.Copy)
        conv9(padB, padA, d1_taps, HD, HD, d1_evict, n0=3)

        d1_int = interior(padB)
        ds1 = small.tile([HD, 1], F32, tag="ds1")
        nc.vector.tensor_reduce(ds1[:], d1_int, axis=X.XY, op=OP.add)
        ds2 = small.tile([HD, 1], F32, tag="ds2")
        nc.scalar.activation(interior(padA), d1_int, AF.Square,
                             accum_out=ds2[:])
        packed = small.tile([HD, 2], F32, tag="pk")
        nc.vector.tensor_copy(packed[:, 0:1], ds1[:])
        nc.vector.tensor_copy(packed[:, 1:2], ds2[:])
        nc.sync.dma_start(cc_in[:], packed[:])
        # 4-way: each group holds all 4 batches exactly once -> exact stats
        nc.gpsimd.collective_compute(
            "AllReduce", OP.add,
            replica_groups=[[0, 1, 2, 3], [4, 5, 6, 7]],
            ins=[cc_in[:]], outs=[cc_out[:]])
        red = small.tile([HD, 2], F32, tag="red")
        nc.sync.dma_start(red[:], cc_out[:])
        sc1, bi1 = bn_scale_bias(red[:, 0:1], red[:, 1:2], B * L,
                                 d1g_t[:], d1be_t[:], "bn1")

        # h2 into padA (pads still zero outside interior; square scratch
        # gets overwritten)
        nc.scalar.activation(interior(padA), d1_int, AF.Relu,
                             bias=bi1[:], scale=sc1[:])
        conv9(out_pad, padA, d2_taps, NFH, NFH,
              lambda d, p: nc.scalar.activation(
                  d, p, AF.Identity, bias=d2b_t[:], scale=1.0))
        out_int = out_pad[:NFH, PBASE : PBASE + PW * H].rearrange(
            "p (h w) -> p h w", w=PW)[:, :, 0:W]
        nc.sync.dma_start(out_ext[:].rearrange("p (h w) -> p h w", w=W),
                          out_int)

    split_excess_waits(nc)
    return nc


_CACHED = {}


def _get_kernel():
    if "nc" not in _CACHED:
        _CACHED["nc"] = build_kernel()
    return _CACHED["nc"]


def _host_inputs(inputs):
    f32 = np.float32
    bf16 = ml_dtypes.bfloat16
    x = np.asarray(inputs["x"], f32)
    enc_w = np.asarray(inputs["enc_w"], f32)
    in_proj = np.asarray(inputs["in_proj"], f32)
    conv_w = np.asarray(inputs["conv_w"], f32)
    x_proj = np.asarray(inputs["x_proj"], f32)
    dt_w = np.asarray(inputs["dt_w"], f32)
    A_log = np.asarray(inputs["A_log"], f32)
    out_proj = np.asarray(inputs["out_proj"], f32)
    dec1_w = np.asarray(inputs["dec1_w"], f32)
    dec2_w = np.asarray(inputs["dec2_w"], f32)

    xp = np.zeros((B, NB, H + 2, W + 2), f32)
    xp[:, :, 1:-1, 1:-1] = x
    cols = np.empty((NB, 3, 3, B, L), f32)
    for dy in range(3):
        for dx in range(3):
            cols[:, dy, dx] = (
                xp[:, :, dy : dy + H, dx : dx + W]
                .reshape(B, NB, L).transpose(1, 0, 2))
    cols_b = cols.reshape(45, B, L)
    enc_w2 = np.ascontiguousarray(enc_w.reshape(HD, 45).T)

    ip_tap = np.empty((HD, NL, DC, DI), f32)
    ip_z = np.empty((HD, NL, DI), f32)
    wd_T = np.empty((DI, NL, DI), f32)
    bc_full = np.empty((DI, NL, 2 * DS), f32)
    a_full = np.empty((DI, NL, DS), f32)
    op_T = np.empty((DI, NL, HD), f32)
    for i in range(NL):
        for k in range(DC):
            ip_tap[:, i, k, :] = (conv_w[i][:, k : k + 1] * in_proj[i][:DI]).T
        ip_z[:, i, :] = in_proj[i][DI:].T
        wd_T[:, i, :] = (dt_w[i] @ x_proj[i][:DTR]).T
        bc_full[:, i, :] = x_proj[i][DTR:].T
        a_full[:, i, :] = -np.exp(A_log[i])   # dA = exp(a_col * delta)
        op_T[:, i, :] = out_proj[i].T

    dec1_tap = np.empty((HD, 9, HD), f32)
    dec2_tap = np.empty((HD, 9, NF), f32)
    for ti in range(9):
        dy, dx = ti // 3, ti % 3
        dec1_tap[:, ti, :] = dec1_w[:, :, dy, dx].T
        dec2_tap[:, ti, :] = dec2_w[:, :, dy, dx].T
    dec2_b_full = np.asarray(inputs["dec2_b"], f32)

    common = {
        "enc_w2": enc_w2.astype(np.float16),
        "enc_g": np.asarray(inputs["enc_g"], f32).reshape(HD, 1),
        "enc_be": np.asarray(inputs["enc_be"], f32).reshape(HD, 1),
        "ip_tap": ip_tap.reshape(HD, NL * DC * DI).astype(np.float16),
        "ip_z": ip_z.reshape(HD, NL * DI).astype(np.float16),
        "conv_b": np.ascontiguousarray(
            np.asarray(inputs["conv_b"], f32).T),           # (DI, NL)
        "wd_T": wd_T.reshape(DI, NL * DI).astype(bf16),

        "dt_b": np.ascontiguousarray(np.asarray(inputs["dt_b"], f32).T),

        "d_col": np.ascontiguousarray(np.asarray(inputs["Dp"], f32).T) / 2.0,
        "op_T": op_T.reshape(DI, NL * HD).astype(bf16),
        "dec1_tap": dec1_tap.reshape(HD, 9 * HD).astype(np.float16),
        "dec1_g": np.asarray(inputs["dec1_g"], f32).reshape(HD, 1),
        "dec1_be": np.asarray(inputs["dec1_be"], f32).reshape(HD, 1),
    }
    in_maps = []
    for c in range(8):
        b0 = c % B
        sr = (c // B) * DSL
        ch0 = (c // B) * NFH
        m = dict(common)
        order = [b0] + [bb for bb in range(B) if bb != b0]
        m["enc_im2col"] = np.ascontiguousarray(
            cols_b[:, order, :].reshape(45, B * L)).astype(np.float16)
        bcs = np.concatenate(
            [bc_full[:, :, sr : sr + DSL],
             bc_full[:, :, DS + sr : DS + sr + DSL]], axis=2)
        m["bc_T"] = np.ascontiguousarray(
            bcs.reshape(DI, NL * 2 * DSL)).astype(bf16)
        m["a_cols"] = np.ascontiguousarray(
            a_full[:, :, sr : sr + DSL].reshape(DI, NL * DSL))
        m["dec2_tap"] = np.ascontiguousarray(
            dec2_tap[:, :, ch0 : ch0 + NFH].reshape(HD, 9 * NFH)
        ).astype(np.float16)
        m["dec2_b"] = np.ascontiguousarray(
            dec2_b_full[ch0 : ch0 + NFH].reshape(NFH, 1))
        in_maps.append(m)
    return in_maps


def kernel(**inputs):
    nc = _get_kernel()
    in_maps = _host_inputs(inputs)
    res = run_bass_kernel_spmd(nc, in_maps, core_ids=list(range(8)))
    out = np.empty((B, NF, H, W), np.float32)
    for b_ in range(B):
        out[b_, :NFH] = np.asarray(
            res.results[b_]["out"], np.float32).reshape(NFH, H, W)
        out[b_, NFH:] = np.asarray(
            res.results[b_ + B]["out"], np.float32).reshape(NFH, H, W)
    return out


if __name__ == "__main__":
    sys.path.insert(0, "/root/problem")
    import reference as ref

    inp = {k: np.asarray(v) for k, v in ref.setup_inputs().items()}
    got = kernel(**inp)
    print("kernel ran, output shape:", got.shape)


# revision 57
# speedup vs baseline: 1.0386x; 1.0222x over previous
"""Trainium2 Bass kernel for MinimalEventMamba.

kernel(**inputs) takes FULL inputs (as from setup_inputs()) and returns the
FULL (4, 10, 64, 64) float32 output. Internally: batch-parallel across 8
NeuronCores (4 batches x2 replicated, state-split across the pair), one SPMD
Bass kernel launch, host assembles the output.

Per-core layout: channel-on-partition, L=4096 on free dim.
- encoder conv as im2col matmul (own batch only; BN stats via 8-way AllReduce)
- mamba trunk: depthwise conv folded into in_proj taps (shifted matmuls,
  PSUM accumulate), dt_w folded into x_proj, Silu/Softplus computed as single
  fused Act ops on PSUM eviction, selective scan via tensor_tensor_scan on
  DVE per state dim, B/C rows broadcast across partitions via DRAM-bounce
  DMA, hc-multiplies offloaded to GpSimd, residual updates on GpSimd.
- per-layer y AllReduce split into two halves pipelined behind the scan
  phase (half-0 collective runs under half-1 scans; half-1 collective runs
  under the next layer's chunk work).
- decoder: dec1 duplicated per pair (cross-batch BN stats via AllReduce),
  dec2 split by output channel across the pair (5+5).
"""
import sys
import types

sys.path.insert(0, "/opt/trn_rl_repo")
sys.path.insert(0, "/opt/trn_rl_repo/concourse")
try:
    from antenv import axon_hooks  # noqa: F401
except ImportError:
    try:
        from trn_agent_boot.trn_boot import _ntff_profile_via_ctypes
        _m = types.ModuleType("antenv.axon_hooks")
        _h = _ntff_profile_via_ctypes("/opt/axon/libaxon_pjrt.so")
        _m.get_axon_ntff_profile_hook = lambda: _h
        _m.set_axon_ntff_profile_hook = lambda h: None
        sys.modules["antenv.axon_hooks"] = _m
    except Exception:
        pass

from contextlib import ExitStack

import numpy as np
import ml_dtypes

import concourse.bass as bass
import concourse.tile as tile
from concourse import mybir
from concourse.bass_utils import run_bass_kernel_spmd
import bass_rust

F32 = mybir.dt.float32
F32R = mybir.dt.float32r
BF16 = mybir.dt.bfloat16
FP16 = mybir.dt.float16

NB, HD, NL, NF = 5, 64, 4, 10
DI, DS, DC, DTR = 128, 16, 4, 4
B, H, W = 4, 64, 64
L = H * W                     # 4096
PW = W + 2                    # padded row stride 66
PADL = PW * (H + 2) + 4       # padded spatial + guard cols (4360)
PBASE = 1 + PW + 1            # first interior col in padded layout
NCHUNK = 8                    # L / 512
CS = 512
LH = L // 2                   # half length (2048)
DSL = DS // 2                 # states per core (s-split across core pairs)
NFH = NF // 2                 # dec2 output channels per core

HC_ON_POOL = False            # GpSimd elementwise measured 4x slower than DVE
                              # 2x mode AND contends for SBUF ports with DVE


def split_excess_waits(nc, max_waits=1):
    """This container's walrus accepts only 1 sync wait per instruction;
    move overflow waits onto NOPs inserted before the offending op."""
    f = nc.m.functions[0]
    for bb in f.blocks:
        insts = bb.instructions
        i = 0
        while i < len(insts):
            inst = insts[i]
            si = inst.sync_info
            if si is not None and len(si.on_wait) > max_waits:
                waits = list(si.on_wait)
                si.on_wait = waits[-max_waits:]
                inst.sync_info = si
                overflow = waits[:-max_waits]
                eng = nc.engines[inst.engine]
                pos = i
                for j in range(0, len(overflow), max_waits):
                    nop = eng.nop(hint="splitw", nofuse=True)
                    nop_inst = nop.ins if hasattr(nop, "ins") else nop
                    for bb2 in f.blocks:
                        if any(x is nop_inst for x in bb2.instructions):
                            bb2.instructions[:] = [
                                x for x in bb2.instructions if x is not nop_inst
                            ]
                            break
                    nop_inst.sync_info = bass_rust.SyncInfo(
                        on_wait=overflow[j : j + max_waits], on_update=[]
                    )
                    insts.insert(pos, nop_inst)
                    pos += 1
                i = pos + 1
            else:
                i += 1


def build_kernel():
    nc = bass.Bass()
    dp = nc.declare_dram_parameter

    enc_in = dp("enc_im2col", [45, B * L], FP16, isOutput=False)
    enc_w2 = dp("enc_w2", [45, HD], FP16, isOutput=False)
    enc_g = dp("enc_g", [HD, 1], F32, isOutput=False)
    enc_be = dp("enc_be", [HD, 1], F32, isOutput=False)
    ip_tap = dp("ip_tap", [HD, NL * DC * DI], FP16, isOutput=False)
    ip_z = dp("ip_z", [HD, NL * DI], FP16, isOutput=False)
    conv_b = dp("conv_b", [DI, NL], F32, isOutput=False)
    wd_T = dp("wd_T", [DI, NL * DI], BF16, isOutput=False)
    bc_T = dp("bc_T", [DI, NL * 2 * DSL], BF16, isOutput=False)
    dt_b = dp("dt_b", [DI, NL], F32, isOutput=False)
    a_cols = dp("a_cols", [DI, NL * DSL], F32, isOutput=False)
    d_col = dp("d_col", [DI, NL], F32, isOutput=False)
    op_T = dp("op_T", [DI, NL * HD], BF16, isOutput=False)
    dec1_tap = dp("dec1_tap", [HD, 9 * HD], FP16, isOutput=False)
    dec1_g = dp("dec1_g", [HD, 1], F32, isOutput=False)
    dec1_be = dp("dec1_be", [HD, 1], F32, isOutput=False)
    dec2_tap = dp("dec2_tap", [HD, 9 * NFH], FP16, isOutput=False)
    dec2_b = dp("dec2_b", [NFH, 1], F32, isOutput=False)

    out_ext = dp("out", [NFH, L], BF16, isOutput=True)

    bc_dram = nc.dram_tensor("bc_dram", [NL, 2 * DSL, L], BF16)
    y_in = [nc.dram_tensor(f"y_in{h}", [HD, LH], BF16) for h in range(2)]
    y_out = [nc.dram_tensor(f"y_out{h}", [HD, LH], BF16) for h in range(2)]
    cc_in = nc.dram_tensor("cc_in", [HD, 2], F32)
    cc_out = nc.dram_tensor("cc_out", [HD, 2], F32)

    ctx = ExitStack()
    with ctx:
        tc = ctx.enter_context(tile.TileContext(nc))
        const = ctx.enter_context(tc.tile_pool(name="const", bufs=1))
        persist = ctx.enter_context(tc.tile_pool(name="persist", bufs=1))
        work = ctx.enter_context(tc.tile_pool(name="work", bufs=1))
        stream = ctx.enter_context(tc.tile_pool(name="stream", bufs=2))
        sloop = ctx.enter_context(tc.tile_pool(name="sloop", bufs=2))
        tail = ctx.enter_context(tc.tile_pool(name="tail", bufs=1))
        small = ctx.enter_context(tc.tile_pool(name="small", bufs=1))
        psum = ctx.enter_context(tc.tile_pool(name="psum", bufs=6, space="PSUM"))

        MM = nc.tensor.matmul
        AF = mybir.ActivationFunctionType
        OP = mybir.AluOpType
        X = mybir.AxisListType
        PAIRS = [[0, 4], [1, 5], [2, 6], [3, 7]]
        ALL8 = [list(range(8))]

        # ------- encoder (all 4 batches locally -> exact BN stats) --------
        enc_w_t = const.tile([45, HD], FP16)
        nc.sync.dma_start(enc_w_t[:], enc_w2[:])
        enc_g_t = const.tile([HD, 1], F32)
        nc.sync.dma_start(enc_g_t[:], enc_g[:])
        enc_be_t = const.tile([HD, 1], F32)
        nc.sync.dma_start(enc_be_t[:], enc_be[:])

        t_t = persist.tile([HD, 4 + L], F32)
        nc.vector.memset(t_t[:, 0:4], 0.0)
        s1p = small.tile([HD, 32], F32, tag="s1p")
        s2p = small.tile([HD, 32], F32, tag="s2p")
        for n in range(32):
            cin = stream.tile([45, CS], FP16, tag="enc_cin")
            nc.sync.dma_start(cin[:], enc_in[:, bass.ts(n, CS)])
            pt = psum.tile([HD, CS], F32, tag="mm512")
            MM(pt[:], enc_w_t[:], cin[:], start=True, stop=True)
            if n < NCHUNK:
                dst = t_t[:, 4 + n * CS : 4 + (n + 1) * CS]
            else:
                scratch = stream.tile([HD, CS], F32, tag="enc_scr")
                dst = scratch[:]
            nc.scalar.activation(dst, pt[:], AF.Copy,
                                 accum_out=s1p[:, n : n + 1])
            sq = stream.tile([HD, CS], F32, tag="enc_scr2")
            nc.scalar.activation(sq[:], pt[:], AF.Square,
                                 accum_out=s2p[:, n : n + 1])
        s1 = small.tile([HD, 1], F32, tag="s1")
        s2 = small.tile([HD, 1], F32, tag="s2")
        nc.vector.tensor_reduce(s1[:], s1p[:], axis=X.X, op=OP.add)
        nc.vector.tensor_reduce(s2[:], s2p[:], axis=X.X, op=OP.add)

        def bn_scale_bias(s1ap, s2ap, n_elems, g_ap, be_ap, tag):
            inv_n = 1.0 / n_elems
            mean = small.tile([HD, 1], F32, tag=tag + "m")
            nc.vector.tensor_scalar_mul(mean[:], s1ap, inv_n)
            m2 = small.tile([HD, 1], F32, tag=tag + "m2")
            nc.vector.tensor_tensor(m2[:], mean[:], mean[:], OP.mult)
            var = small.tile([HD, 1], F32, tag=tag + "v")
            nc.vector.scalar_tensor_tensor(var[:], s2ap, inv_n, m2[:],
                                           OP.mult, OP.subtract)
            veps = small.tile([HD, 1], F32, tag=tag + "ve")
            nc.vector.tensor_scalar_add(veps[:], var[:], 1e-5)
            rv = small.tile([HD, 1], F32, tag=tag + "rv")
            nc.vector.reciprocal(rv[:], veps[:])
            rstd = small.tile([HD, 1], F32, tag=tag + "rs")
            nc.scalar.activation(rstd[:], rv[:], AF.Sqrt)
            scale = small.tile([HD, 1], F32, tag=tag + "sc")
            nc.vector.tensor_tensor(scale[:], g_ap, rstd[:], OP.mult)
            nscale = small.tile([HD, 1], F32, tag=tag + "ns")
            nc.vector.tensor_scalar_mul(nscale[:], scale[:], -1.0)
            bias = small.tile([HD, 1], F32, tag=tag + "bi")
            nc.vector.scalar_tensor_tensor(bias[:], mean[:], nscale[:], be_ap,
                                           OP.mult, OP.add)
            return scale, bias

        sc0, bi0 = bn_scale_bias(s1[:], s2[:], B * L,
                                 enc_g_t[:], enc_be_t[:], "bn0")

        nc.scalar.activation(t_t[:, 4:], t_t[:, 4:], AF.Relu,
                             bias=bi0[:], scale=sc0[:])
        t16 = persist.tile([HD, 4 + L], FP16)
        nc.vector.memset(t16[:, 0:4], 0.0)
        nc.scalar.activation(t16[:, 4:], t_t[:, 4:], AF.Copy)

        # ---------------- trunk weights ----------------
        iptap_t = const.tile([HD, NL * DC * DI], FP16)
        nc.sync.dma_start(iptap_t[:], ip_tap[:])
        ipz_t = const.tile([HD, NL * DI], FP16)
        nc.sync.dma_start(ipz_t[:], ip_z[:])
        convb_t = const.tile([DI, NL], F32)
        nc.sync.dma_start(convb_t[:], conv_b[:])
        wd_t = const.tile([DI, NL * DI], BF16)
        nc.sync.dma_start(wd_t[:], wd_T[:])
        bct_t = const.tile([DI, NL * 2 * DSL], BF16)
        nc.sync.dma_start(bct_t[:], bc_T[:])
        dtb_t = const.tile([DI, NL], F32)
        nc.sync.dma_start(dtb_t[:], dt_b[:])
        acols_t = const.tile([DI, NL * DSL], F32)
        nc.sync.dma_start(acols_t[:], a_cols[:])
        dcol_t = const.tile([DI, NL], F32)
        nc.sync.dma_start(dcol_t[:], d_col[:])
        opt_t = const.tile([DI, NL * HD], BF16)
        nc.sync.dma_start(opt_t[:], op_T[:])

        hmid = persist.tile([DI, DSL], F32)

        # ---------------- trunk (software-pipelined) ----------------
        # Per layer: [1a 2a] issued by previous iteration; s0 | 1b 2b |
        # tail0(coll#1) | s1 | 1a'(next) 2a'(next) | tail1(coll#2).
        # Residual updates (DVE) are injected mid-s-phase so their semaphore
        # waits never head-block the DVE queue.
        INJ = 5   # inject deferred t-updates after this many scan states

        def mk_layer(li):
            xi_c = work.tile([DI, L], BF16, tag="xi_c")
            sz = work.tile([DI, L], BF16, tag="sz")
            dlt = work.tile([DI, L], BF16, tag="dlt")
            bc_sb = work.tile([2 * DSL, L], BF16, tag="bc_sb")
            du = work.tile([DI, L], BF16, tag="du")
            ly = {"li": li, "xi_c": xi_c, "sz": sz, "dlt": dlt,
                  "bc_sb": bc_sb, "du": du}

            def chunk1(n):
                p_xi = psum.tile([DI, CS], F32, tag="mm512")
                for k in range(DC):
                    MM(p_xi[:],
                       iptap_t[:, (li * DC + k) * DI : (li * DC + k + 1) * DI],
                       t16[:, 1 + k + n * CS : 1 + k + n * CS + CS],
                       start=(k == 0), stop=(k == DC - 1))
                nc.scalar.activation(ly["xi_c"][:, bass.ts(n, CS)], p_xi[:],
                                     AF.Silu,
                                     bias=convb_t[:, li : li + 1], scale=1.0)
                p_z = psum.tile([DI, CS], F32, tag="mm512")
                MM(p_z[:], ipz_t[:, li * DI : (li + 1) * DI],
                   t16[:, 4 + n * CS : 4 + (n + 1) * CS],
                   start=True, stop=True)
                nc.scalar.activation(ly["sz"][:, bass.ts(n, CS)], p_z[:],
                                     AF.Silu)

            def chunk2(n, ev):
                p_d = psum.tile([DI, CS], F32, tag="mm512")
                MM(p_d[:], wd_t[:, li * DI : (li + 1) * DI],
                   ly["xi_c"][:, bass.ts(n, CS)], start=True, stop=True)
                # ev = exp(p_d + dt_b); delta = ln(1 + ev) applied in s_half.
                # Exp/Ln share the natural_log_exp act table -> no reloads.
                nc.scalar.activation(ev[:, bass.ts(n % 4, CS)], p_d[:],
                                     AF.Exp,
                                     bias=dtb_t[:, li : li + 1], scale=1.0)
                p_bc = psum.tile([2 * DSL, CS], F32, tag="mm512")
                MM(p_bc[:], bct_t[:, li * 2 * DSL : (li + 1) * 2 * DSL],
                   ly["xi_c"][:, bass.ts(n, CS)], start=True, stop=True)
                nc.scalar.activation(ly["bc_sb"][:, bass.ts(n, CS)], p_bc[:],
                                     AF.Copy)

            def half_chunks(hf):
                for n in range(hf * 4, hf * 4 + 4):
                    chunk1(n)
                ev = tail.tile([DI, LH], F32, tag=f"ev{hf}")
                for n in range(hf * 4, hf * 4 + 4):
                    chunk2(n, ev)
                sl = slice(hf * LH, (hf + 1) * LH)
                nc.sync.dma_start(bc_dram[li][:, sl], ly["bc_sb"][:, sl])
                return ev

            def s_half(hf, ev, inject=None):
                sl = slice(hf * LH, (hf + 1) * LH)
                dlt, du, xi_c = ly["dlt"], ly["du"], ly["xi_c"]
                nc.scalar.activation(dlt[:, sl], ev[:], AF.Ln, bias=1.0,
                                     scale=1.0)
                nc.vector.tensor_tensor(du[:, sl], dlt[:, sl], xi_c[:, sl],
                                        OP.mult)
                acc = None
                pend = None
                for s in range(DSL):
                    if s == INJ and inject is not None:
                        inject()
                    dA = sloop.tile([DI, LH], FP16, tag="dA")
                    nc.scalar.activation(
                        dA[:], dlt[:, sl], AF.Exp,
                        scale=acols_t[:, li * DSL + s : li * DSL + s + 1])
                    brep = sloop.tile([DI, LH], BF16, tag="brep")
                    nc.sync.dma_start(
                        brep[:],
                        bc_dram[li][s : s + 1, sl].broadcast_to((DI, LH)))
                    crep = sloop.tile([DI, LH], BF16, tag="crep")
                    nc.sync.dma_start(
                        crep[:],
                        bc_dram[li][DSL + s : DSL + s + 1, sl].broadcast_to(
                            (DI, LH)))
                    xs = sloop.tile([DI, LH], BF16, tag="xs")
                    nc.vector.tensor_tensor(xs[:], du[:, sl], brep[:], OP.mult)
                    hs = sloop.tile([DI, LH], BF16, tag="hs")
                    init = 0.0 if hf == 0 else hmid[:, s : s + 1]
                    nc.vector.tensor_tensor_scan(hs[:], dA[:], xs[:], init,
                                                 OP.mult, OP.add)
                    if hf == 0:
                        nc.vector.tensor_copy(hmid[:, s : s + 1],
                                              hs[:, LH - 1 : LH])
                    if pend is not None:
                        anew = sloop.tile([DI, LH], BF16, tag=f"acc{hf}")
                        if acc is None:
                            # fold the D*u term into the first accumulate
                            nc.vector.scalar_tensor_tensor(
                                anew[:], xi_c[:, sl],
                                dcol_t[:, li : li + 1], pend[:],
                                OP.mult, OP.add)
                        else:
                            nc.vector.tensor_tensor(anew[:], acc[:], pend[:],
                                                    OP.add)
                        acc = anew
                    hc = sloop.tile([DI, LH], BF16, tag="hc")
                    nc.vector.tensor_tensor(hc[:], hs[:], crep[:], OP.mult)
                    pend = hc
                anew = sloop.tile([DI, LH], BF16, tag=f"acc{hf}")
                nc.vector.tensor_tensor(anew[:], acc[:], pend[:], OP.add)
                return anew

            def half_tail(hf, acc):
                sl = slice(hf * LH, (hf + 1) * LH)
                yg = tail.tile([DI, LH], BF16, tag=f"yg{hf}")
                nc.vector.tensor_tensor(yg[:], acc[:], ly["sz"][:, sl],
                                        OP.mult)
                dt_part = tail.tile([HD, LH], BF16, tag=f"dtp{hf}")
                for n in range(4):
                    p_o = psum.tile([HD, CS], F32, tag="mm512")
                    MM(p_o[:], opt_t[:, li * HD : (li + 1) * HD],
                       yg[:, bass.ts(n, CS)], start=True, stop=True)
                    nc.scalar.activation(dt_part[:, bass.ts(n, CS)], p_o[:],
                                         AF.Copy)
                nc.sync.dma_start(y_in[hf][:], dt_part[:])
                nc.gpsimd.collective_compute(
                    "AllReduce", OP.add, replica_groups=PAIRS,
                    ins=[y_in[hf][:]], outs=[y_out[hf][:]])

            def half_update(hf):
                sl = slice(4 + hf * LH, 4 + (hf + 1) * LH)
                # reuse dtp slot: its DMA into y_in completes before the
                # collective can produce y_out
                dtsum = tail.tile([HD, LH], BF16, tag=f"dtp{hf}")
                nc.sync.dma_start(dtsum[:], y_out[hf][:])
                nc.vector.tensor_tensor(t_t[:, sl], t_t[:, sl], dtsum[:],
                                        OP.add)
                if li < NL - 1:
                    nc.vector.tensor_copy(t16[:, sl], t_t[:, sl])

            ly.update(chunk1=chunk1, chunk2=chunk2, half_chunks=half_chunks,
                      s_half=s_half, half_tail=half_tail,
                      half_update=half_update)
            return ly

        cur = mk_layer(0)
        ev0 = cur["half_chunks"](0)
        pend_update = None
        for li in range(NL):
            acc0 = cur["s_half"](0, ev0, inject=pend_update)
            ev1 = cur["half_chunks"](1)
            cur["half_tail"](0, acc0)
            hu = cur["half_update"]
            acc1 = cur["s_half"](1, ev1, inject=lambda hu=hu: hu(0))
            if li + 1 < NL:
                nxt = mk_layer(li + 1)
                ev0 = nxt["half_chunks"](0)
            cur["half_tail"](1, acc1)
            pend_update = lambda hu=hu: hu(1)
            if li + 1 < NL:
                cur = nxt

        # ---------------- decoder ----------------
        d1_taps = const.tile([HD, 9 * HD], FP16)
        nc.sync.dma_start(d1_taps[:], dec1_tap[:])
        d2_taps = const.tile([HD, 9 * NFH], FP16)
        nc.sync.dma_start(d2_taps[:], dec2_tap[:])
        d1g_t = const.tile([HD, 1], F32)
        nc.sync.dma_start(d1g_t[:], dec1_g[:])
        d1be_t = const.tile([HD, 1], F32)
        nc.sync.dma_start(d1be_t[:], dec1_be[:])
        d2b_t = const.tile([NFH, 1], F32)
        nc.sync.dma_start(d2b_t[:], dec2_b[:])
        padA = persist.tile([HD, PADL], FP16)
        nc.vector.memset(padA[:], 0.0)
        padB = persist.tile([HD, PADL], FP16)
        nc.vector.memset(padB[:], 0.0)
        out_pad = persist.tile([NFH, PADL], BF16)

        def interior(tile_ap):
            return tile_ap[:, PBASE : PBASE + PW * H].rearrange(
                "p (h w) -> p h w", w=PW)[:, :, 0:W]

        # top half of the image is final before the last half-1 residual
        # update; start dec1 on it while the last y-collective completes
        nc.scalar.activation(
            interior(padA)[:, 0:32, :],
            t_t[:, 4 : 4 + LH].rearrange("p (h w) -> p h w", w=W),
            AF.Copy)

        def conv9(dst_tile, src_tile, taps_tile, m_out, tapw, evict,
                  n0=0, n1=None):
            total = PW * H
            nch = (total + CS - 1) // CS
            for n in range(n0, nch if n1 is None else n1):
                c0 = PBASE + n * CS
                cw = min(CS, PBASE + total - c0)
                pt = psum.tile([m_out, CS], F32, tag="mm512")
                for ti in range(9):
                    dy, dx = ti // 3, ti % 3
                    off = c0 + (dy - 1) * PW + (dx - 1)
                    MM(pt[:, 0:cw],
                       taps_tile[:, ti * tapw : ti * tapw + m_out],
                       src_tile[:, off : off + cw],
                       start=(ti == 0), stop=(ti == 8))
                evict(dst_tile[0:m_out, c0 : c0 + cw], pt[:, 0:cw])

        d1_evict = lambda d, p: nc.scalar.activation(d, p, AF.Copy)
        conv9(padB, padA, d1_taps, HD, HD, d1_evict, n0=0, n1=3)
        pend_update()       # last layer's half-1 residual into t_t
        nc.scalar.activation(
            interior(padA)[:, 32:64, :],
            t_t[:, 4 + LH :].rearrange("p (h w) -> p h w", w=W),
            AF.Copy)
        conv9(padB, padA, d1_taps, HD, HD, d1_evict, n0=3)

        d1_int = interior(padB)
        ds1 = small.tile([HD, 1], F32, tag="ds1")
        nc.vector.tensor_reduce(ds1[:], d1_int, axis=X.XY, op=OP.add)
        ds2 = small.tile([HD, 1], F32, tag="ds2")
        nc.scalar.activation(interior(padA), d1_int, AF.Square,
                             accum_out=ds2[:])
        packed = small.tile([HD, 2], F32, tag="pk")
        nc.vector.tensor_copy(packed[:, 0:1], ds1[:])
        nc.vector.tensor_copy(packed[:, 1:2], ds2[:])
        nc.sync.dma_start(cc_in[:], packed[:])
        # 4-way: each group holds all 4 batches exactly once -> exact stats
        nc.gpsimd.collective_compute(
            "AllReduce", OP.add,
            replica_groups=[[0, 1, 2, 3], [4, 5, 6, 7]],
            ins=[cc_in[:]], outs=[cc_out[:]])
        red = small.tile([HD, 2], F32, tag="red")
        nc.sync.dma_start(red[:], cc_out[:])
        sc1, bi1 = bn_scale_bias(red[:, 0:1], red[:, 1:2], B * L,
                                 d1g_t[:], d1be_t[:], "bn1")

        # h2 into padA (pads still zero outside interior; square scratch
        # gets overwritten)
        nc.scalar.activation(interior(padA), d1_int, AF.Relu,
                             bias=bi1[:], scale=sc1[:])
        conv9(out_pad, padA, d2_taps, NFH, NFH,
              lambda d, p: nc.scalar.activation(
                  d, p, AF.Identity, bias=d2b_t[:], scale=1.0))
        out_int = out_pad[:NFH, PBASE : PBASE + PW * H].rearrange(
            "p (h w) -> p h w", w=PW)[:, :, 0:W]
        nc.sync.dma_start(out_ext[:].rearrange("p (h w) -> p h w", w=W),
                          out_int)

    split_excess_waits(nc)
    return nc


_CACHED = {}


def _get_kernel():
    if "nc" not in _CACHED:
        _CACHED["nc"] = build_kernel()
    return _CACHED["nc"]


def _host_inputs(inputs):
    f32 = np.float32
    bf16 = ml_dtypes.bfloat16
    x = np.asarray(inputs["x"], f32)
    enc_w = np.asarray(inputs["enc_w"], f32)
    in_proj = np.asarray(inputs["in_proj"], f32)
    conv_w = np.asarray(inputs["conv_w"], f32)
    x_proj = np.asarray(inputs["x_proj"], f32)
    dt_w = np.asarray(inputs["dt_w"], f32)
    A_log = np.asarray(inputs["A_log"], f32)
    out_proj = np.asarray(inputs["out_proj"], f32)
    dec1_w = np.asarray(inputs["dec1_w"], f32)
    dec2_w = np.asarray(inputs["dec2_w"], f32)

    xp = np.zeros((B, NB, H + 2, W + 2), f32)
    xp[:, :, 1:-1, 1:-1] = x
    cols = np.empty((NB, 3, 3, B, L), f32)
    for dy in range(3):
        for dx in range(3):
            cols[:, dy, dx] = (
                xp[:, :, dy : dy + H, dx : dx + W]
                .reshape(B, NB, L).transpose(1, 0, 2))
    cols_b = cols.reshape(45, B, L)
    enc_w2 = np.ascontiguousarray(enc_w.reshape(HD, 45).T)

    ip_tap = np.empty((HD, NL, DC, DI), f32)
    ip_z = np.empty((HD, NL, DI), f32)
    wd_T = np.empty((DI, NL, DI), f32)
    bc_full = np.empty((DI, NL, 2 * DS), f32)
    a_full = np.empty((DI, NL, DS), f32)
    op_T = np.empty((DI, NL, HD), f32)
    for i in range(NL):
        for k in range(DC):
            ip_tap[:, i, k, :] = (conv_w[i][:, k : k + 1] * in_proj[i][:DI]).T
        ip_z[:, i, :] = in_proj[i][DI:].T
        wd_T[:, i, :] = (dt_w[i] @ x_proj[i][:DTR]).T
        bc_full[:, i, :] = x_proj[i][DTR:].T
        a_full[:, i, :] = -np.exp(A_log[i])   # dA = exp(a_col * delta)
        op_T[:, i, :] = out_proj[i].T

    dec1_tap = np.empty((HD, 9, HD), f32)
    dec2_tap = np.empty((HD, 9, NF), f32)
    for ti in range(9):
        dy, dx = ti // 3, ti % 3
        dec1_tap[:, ti, :] = dec1_w[:, :, dy, dx].T
        dec2_tap[:, ti, :] = dec2_w[:, :, dy, dx].T
    dec2_b_full = np.asarray(inputs["dec2_b"], f32)

    common = {
        "enc_w2": enc_w2.astype(np.float16),
        "enc_g": np.asarray(inputs["enc_g"], f32).reshape(HD, 1),
        "enc_be": np.asarray(inputs["enc_be"], f32).reshape(HD, 1),
        "ip_tap": ip_tap.reshape(HD, NL * DC * DI).astype(np.float16),
        "ip_z": ip_z.reshape(HD, NL * DI).astype(np.float16),
        "conv_b": np.ascontiguousarray(
            np.asarray(inputs["conv_b"], f32).T),           # (DI, NL)
        "wd_T": wd_T.reshape(DI, NL * DI).astype(bf16),

        "dt_b": np.ascontiguousarray(np.asarray(inputs["dt_b"], f32).T),

        "d_col": np.ascontiguousarray(np.asarray(inputs["Dp"], f32).T) / 2.0,
        "op_T": op_T.reshape(DI, NL * HD).astype(bf16),
        "dec1_tap": dec1_tap.reshape(HD, 9 * HD).astype(np.float16),
        "dec1_g": np.asarray(inputs["dec1_g"], f32).reshape(HD, 1),
        "dec1_be": np.asarray(inputs["dec1_be"], f32).reshape(HD, 1),
    }
    in_maps = []
    for c in range(8):
        b0 = c % B
        sr = (c // B) * DSL
        ch0 = (c // B) * NFH
        m = dict(common)
        order = [b0] + [bb for bb in range(B) if bb != b0]
        m["enc_im2col"] = np.ascontiguousarray(
            cols_b[:, order, :].reshape(45, B * L)).astype(np.float16)
        bcs = np.concatenate(
            [bc_full[:, :, sr : sr + DSL],
             bc_full[:, :, DS + sr : DS + sr + DSL]], axis=2)
        m["bc_T"] = np.ascontiguousarray(
            bcs.reshape(DI, NL * 2 * DSL)).astype(bf16)
        m["a_cols"] = np.ascontiguousarray(
            a_full[:, :, sr : sr + DSL].reshape(DI, NL * DSL))
        m["dec2_tap"] = np.ascontiguousarray(
            dec2_tap[:, :, ch0 : ch0 + NFH].reshape(HD, 9 * NFH)
        ).astype(np.float16)
        m["dec2_b"] = np.ascontiguousarray(
            dec2_b_full[ch0 : ch0 + NFH].reshape(NFH, 1))
        in_maps.append(m)
    return in_maps


def kernel(**inputs):
    nc = _get_kernel()
    in_maps = _host_inputs(inputs)
    res = run_bass_kernel_spmd(nc, in_maps, core_ids=list(range(8)))
    out = np.empty((B, NF, H, W), np.float32)
    for b_ in range(B):
        out[b_, :NFH] = np.asarray(
            res.results[b_]["out"], np.float32).reshape(NFH, H, W)
        out[b_, NFH:] = np.asarray(
            res.results[b_ + B]["out"], np.float32).reshape(NFH, H, W)
    return out


if __name__ == "__main__":
    sys.path.insert(0, "/root/problem")
    import reference as ref

    inp = {k: np.asarray(v) for k, v in ref.setup_inputs().items()}
    got = kernel(**inp)
    print("kernel ran, output shape:", got.shape)


# revision 59
# speedup vs baseline: 1.0494x; 1.0103x over previous
"""Trainium2 Bass kernel for MinimalEventMamba.

kernel(**inputs) takes FULL inputs (as from setup_inputs()) and returns the
FULL (4, 10, 64, 64) float32 output. Internally: batch-parallel across 8
NeuronCores (4 batches x2 replicated, state-split across the pair), one SPMD
Bass kernel launch, host assembles the output.

Per-core layout: channel-on-partition, L=4096 on free dim.
- encoder conv as im2col matmul (own batch only; BN stats via 8-way AllReduce)
- mamba trunk: depthwise conv folded into in_proj taps (shifted matmuls,
  PSUM accumulate), dt_w folded into x_proj, Silu/Softplus computed as single
  fused Act ops on PSUM eviction, selective scan via tensor_tensor_scan on
  DVE per state dim, B/C rows broadcast across partitions via DRAM-bounce
  DMA, hc-multiplies offloaded to GpSimd, residual updates on GpSimd.
- per-layer y AllReduce split into two halves pipelined behind the scan
  phase (half-0 collective runs under half-1 scans; half-1 collective runs
  under the next layer's chunk work).
- decoder: dec1 duplicated per pair (cross-batch BN stats via AllReduce),
  dec2 split by output channel across the pair (5+5).
"""
import sys
import types

sys.path.insert(0, "/opt/trn_rl_repo")
sys.path.insert(0, "/opt/trn_rl_repo/concourse")
try:
    from antenv import axon_hooks  # noqa: F401
except ImportError:
    try:
        from trn_agent_boot.trn_boot import _ntff_profile_via_ctypes
        _m = types.ModuleType("antenv.axon_hooks")
        _h = _ntff_profile_via_ctypes("/opt/axon/libaxon_pjrt.so")
        _m.get_axon_ntff_profile_hook = lambda: _h
        _m.set_axon_ntff_profile_hook = lambda h: None
        sys.modules["antenv.axon_hooks"] = _m
    except Exception:
        pass

from contextlib import ExitStack

import numpy as np
import ml_dtypes

import concourse.bass as bass
import concourse.tile as tile
from concourse import mybir
from concourse.bass_utils import run_bass_kernel_spmd
import bass_rust

F32 = mybir.dt.float32
F32R = mybir.dt.float32r
BF16 = mybir.dt.bfloat16
FP16 = mybir.dt.float16

NB, HD, NL, NF = 5, 64, 4, 10
DI, DS, DC, DTR = 128, 16, 4, 4
B, H, W = 4, 64, 64
L = H * W                     # 4096
PW = W + 2                    # padded row stride 66
PADL = PW * (H + 2) + 4       # padded spatial + guard cols (4360)
PBASE = 1 + PW + 1            # first interior col in padded layout
NCHUNK = 8                    # L / 512
CS = 512
LH = L // 2                   # half length (2048)
DSL = DS // 2                 # states per core (s-split across core pairs)
NFH = NF // 2                 # dec2 output channels per core

HC_ON_POOL = False            # GpSimd elementwise measured 4x slower than DVE
                              # 2x mode AND contends for SBUF ports with DVE


def split_excess_waits(nc, max_waits=1):
    """This container's walrus accepts only 1 sync wait per instruction;
    move overflow waits onto NOPs inserted before the offending op."""
    f = nc.m.functions[0]
    for bb in f.blocks:
        insts = bb.instructions
        i = 0
        while i < len(insts):
            inst = insts[i]
            si = inst.sync_info
            if si is not None and len(si.on_wait) > max_waits:
                waits = list(si.on_wait)
                si.on_wait = waits[-max_waits:]
                inst.sync_info = si
                overflow = waits[:-max_waits]
                eng = nc.engines[inst.engine]
                pos = i
                for j in range(0, len(overflow), max_waits):
                    nop = eng.nop(hint="splitw", nofuse=True)
                    nop_inst = nop.ins if hasattr(nop, "ins") else nop
                    for bb2 in f.blocks:
                        if any(x is nop_inst for x in bb2.instructions):
                            bb2.instructions[:] = [
                                x for x in bb2.instructions if x is not nop_inst
                            ]
                            break
                    nop_inst.sync_info = bass_rust.SyncInfo(
                        on_wait=overflow[j : j + max_waits], on_update=[]
                    )
                    insts.insert(pos, nop_inst)
                    pos += 1
                i = pos + 1
            else:
                i += 1


def build_kernel():
    nc = bass.Bass()
    dp = nc.declare_dram_parameter

    enc_in = dp("enc_im2col", [45, B * L], FP16, isOutput=False)
    enc_w2 = dp("enc_w2", [45, HD], FP16, isOutput=False)
    enc_g = dp("enc_g", [HD, 1], F32, isOutput=False)
    enc_be = dp("enc_be", [HD, 1], F32, isOutput=False)
    ip_tap = dp("ip_tap", [HD, NL * DC * DI], FP16, isOutput=False)
    ip_z = dp("ip_z", [HD, NL * DI], FP16, isOutput=False)
    conv_b = dp("conv_b", [DI, NL], F32, isOutput=False)
    wd_T = dp("wd_T", [DI, NL * DI], BF16, isOutput=False)
    bc_T = dp("bc_T", [DI, NL * 2 * DSL], BF16, isOutput=False)
    dt_b = dp("dt_b", [DI, NL], F32, isOutput=False)
    a_cols = dp("a_cols", [DI, NL * DSL], F32, isOutput=False)
    d_col = dp("d_col", [DI, NL], F32, isOutput=False)
    op_T = dp("op_T", [DI, NL * HD], BF16, isOutput=False)
    dec1_tap = dp("dec1_tap", [HD, 9 * HD], FP16, isOutput=False)
    dec1_g = dp("dec1_g", [HD, 1], F32, isOutput=False)
    dec1_be = dp("dec1_be", [HD, 1], F32, isOutput=False)
    dec2_tap = dp("dec2_tap", [HD, 9 * NFH], FP16, isOutput=False)
    dec2_b = dp("dec2_b", [NFH, 1], F32, isOutput=False)

    out_ext = dp("out", [NFH, L], BF16, isOutput=True)

    bc_dram = nc.dram_tensor("bc_dram", [NL, 2 * DSL, L], BF16)
    y_in = [nc.dram_tensor(f"y_in{h}", [HD, LH], BF16) for h in range(2)]
    y_out = [nc.dram_tensor(f"y_out{h}", [HD, LH], BF16) for h in range(2)]
    cc_in = nc.dram_tensor("cc_in", [HD, 2], F32)
    cc_out = nc.dram_tensor("cc_out", [HD, 2], F32)

    ctx = ExitStack()
    with ctx:
        tc = ctx.enter_context(tile.TileContext(nc))
        const = ctx.enter_context(tc.tile_pool(name="const", bufs=1))
        persist = ctx.enter_context(tc.tile_pool(name="persist", bufs=1))
        work = ctx.enter_context(tc.tile_pool(name="work", bufs=1))
        stream = ctx.enter_context(tc.tile_pool(name="stream", bufs=2))
        sloop = ctx.enter_context(tc.tile_pool(name="sloop", bufs=2))
        tail = ctx.enter_context(tc.tile_pool(name="tail", bufs=1))
        small = ctx.enter_context(tc.tile_pool(name="small", bufs=1))
        psum = ctx.enter_context(tc.tile_pool(name="psum", bufs=6, space="PSUM"))

        MM = nc.tensor.matmul
        AF = mybir.ActivationFunctionType
        OP = mybir.AluOpType
        X = mybir.AxisListType
        PAIRS = [[0, 4], [1, 5], [2, 6], [3, 7]]
        ALL8 = [list(range(8))]

        # ------- encoder (all 4 batches locally -> exact BN stats) --------
        enc_w_t = const.tile([45, HD], FP16)
        nc.sync.dma_start(enc_w_t[:], enc_w2[:])
        enc_g_t = const.tile([HD, 1], F32)
        nc.sync.dma_start(enc_g_t[:], enc_g[:])
        enc_be_t = const.tile([HD, 1], F32)
        nc.sync.dma_start(enc_be_t[:], enc_be[:])

        t_t = persist.tile([HD, 4 + L], F32)
        nc.vector.memset(t_t[:, 0:4], 0.0)
        s1p = small.tile([HD, 32], F32, tag="s1p")
        s2p = small.tile([HD, 32], F32, tag="s2p")
        for n in range(32):
            cin = stream.tile([45, CS], FP16, tag="enc_cin")
            nc.sync.dma_start(cin[:], enc_in[:, bass.ts(n, CS)])
            pt = psum.tile([HD, CS], F32, tag="mm512")
            MM(pt[:], enc_w_t[:], cin[:], start=True, stop=True)
            if n < NCHUNK:
                dst = t_t[:, 4 + n * CS : 4 + (n + 1) * CS]
            else:
                scratch = stream.tile([HD, CS], F32, tag="enc_scr")
                dst = scratch[:]
            if n % 2 == 0:
                # even chunks: Act evicts + accumulates both stats
                nc.scalar.activation(dst, pt[:], AF.Copy,
                                     accum_out=s1p[:, n : n + 1])
                sq = stream.tile([HD, CS], F32, tag="enc_scr2")
                nc.scalar.activation(sq[:], pt[:], AF.Square,
                                     accum_out=s2p[:, n : n + 1])
            else:
                # odd chunks: DVE path — halves the Act-serialized encoder
                nc.vector.tensor_copy(dst, pt[:])
                nc.vector.tensor_reduce(s1p[:, n : n + 1], dst, axis=X.X,
                                        op=OP.add)
                sq = stream.tile([HD, CS], F32, tag="enc_scr2")
                nc.vector.scalar_tensor_tensor(
                    sq[:], dst, 1.0, dst, OP.mult, OP.mult,
                    accum_out=s2p[:, n : n + 1])
        s1 = small.tile([HD, 1], F32, tag="s1")
        s2 = small.tile([HD, 1], F32, tag="s2")
        nc.vector.tensor_reduce(s1[:], s1p[:], axis=X.X, op=OP.add)
        nc.vector.tensor_reduce(s2[:], s2p[:], axis=X.X, op=OP.add)

        def bn_scale_bias(s1ap, s2ap, n_elems, g_ap, be_ap, tag):
            inv_n = 1.0 / n_elems
            mean = small.tile([HD, 1], F32, tag=tag + "m")
            nc.vector.tensor_scalar_mul(mean[:], s1ap, inv_n)
            m2 = small.tile([HD, 1], F32, tag=tag + "m2")
            nc.vector.tensor_tensor(m2[:], mean[:], mean[:], OP.mult)
            var = small.tile([HD, 1], F32, tag=tag + "v")
            nc.vector.scalar_tensor_tensor(var[:], s2ap, inv_n, m2[:],
                                           OP.mult, OP.subtract)
            veps = small.tile([HD, 1], F32, tag=tag + "ve")
            nc.vector.tensor_scalar_add(veps[:], var[:], 1e-5)
            rv = small.tile([HD, 1], F32, tag=tag + "rv")
            nc.vector.reciprocal(rv[:], veps[:])
            rstd = small.tile([HD, 1], F32, tag=tag + "rs")
            nc.scalar.activation(rstd[:], rv[:], AF.Sqrt)
            scale = small.tile([HD, 1], F32, tag=tag + "sc")
            nc.vector.tensor_tensor(scale[:], g_ap, rstd[:], OP.mult)
            nscale = small.tile([HD, 1], F32, tag=tag + "ns")
            nc.vector.tensor_scalar_mul(nscale[:], scale[:], -1.0)
            bias = small.tile([HD, 1], F32, tag=tag + "bi")
            nc.vector.scalar_tensor_tensor(bias[:], mean[:], nscale[:], be_ap,
                                           OP.mult, OP.add)
            return scale, bias

        sc0, bi0 = bn_scale_bias(s1[:], s2[:], B * L,
                                 enc_g_t[:], enc_be_t[:], "bn0")

        # chunked relu+cast so layer 0's first matmuls start after 512 cols
        t16 = persist.tile([HD, 4 + L], FP16)
        nc.vector.memset(t16[:, 0:4], 0.0)
        for n in range(NCHUNK):
            sl = slice(4 + n * CS, 4 + (n + 1) * CS)
            nc.scalar.activation(t_t[:, sl], t_t[:, sl], AF.Relu,
                                 bias=bi0[:], scale=sc0[:])
            nc.scalar.activation(t16[:, sl], t_t[:, sl], AF.Copy)

        # ---------------- trunk weights ----------------
        iptap_t = const.tile([HD, NL * DC * DI], FP16)
        nc.sync.dma_start(iptap_t[:], ip_tap[:])
        ipz_t = const.tile([HD, NL * DI], FP16)
        nc.sync.dma_start(ipz_t[:], ip_z[:])
        convb_t = const.tile([DI, NL], F32)
        nc.sync.dma_start(convb_t[:], conv_b[:])
        wd_t = const.tile([DI, NL * DI], BF16)
        nc.sync.dma_start(wd_t[:], wd_T[:])
        bct_t = const.tile([DI, NL * 2 * DSL], BF16)
        nc.sync.dma_start(bct_t[:], bc_T[:])
        dtb_t = const.tile([DI, NL], F32)
        nc.sync.dma_start(dtb_t[:], dt_b[:])
        acols_t = const.tile([DI, NL * DSL], F32)
        nc.sync.dma_start(acols_t[:], a_cols[:])
        dcol_t = const.tile([DI, NL], F32)
        nc.sync.dma_start(dcol_t[:], d_col[:])
        opt_t = const.tile([DI, NL * HD], BF16)
        nc.sync.dma_start(opt_t[:], op_T[:])

        hmid = persist.tile([DI, DSL], F32)

        # ---------------- trunk (software-pipelined) ----------------
        # Per layer: [1a 2a] issued by previous iteration; s0 | 1b 2b |
        # tail0(coll#1) | s1 | 1a'(next) 2a'(next) | tail1(coll#2).
        # Residual updates (DVE) are injected mid-s-phase so their semaphore
        # waits never head-block the DVE queue.
        INJ = 5   # inject deferred t-updates after this many scan states

        def mk_layer(li):
            xi_c = work.tile([DI, L], BF16, tag="xi_c")
            sz = work.tile([DI, L], BF16, tag="sz")
            dlt = work.tile([DI, L], BF16, tag="dlt")
            bc_sb = work.tile([2 * DSL, L], BF16, tag="bc_sb")
            du = work.tile([DI, L], BF16, tag="du")
            ly = {"li": li, "xi_c": xi_c, "sz": sz, "dlt": dlt,
                  "bc_sb": bc_sb, "du": du}

            def chunk1(n):
                p_xi = psum.tile([DI, CS], F32, tag="mm512")
                for k in range(DC):
                    MM(p_xi[:],
                       iptap_t[:, (li * DC + k) * DI : (li * DC + k + 1) * DI],
                       t16[:, 1 + k + n * CS : 1 + k + n * CS + CS],
                       start=(k == 0), stop=(k == DC - 1))
                nc.scalar.activation(ly["xi_c"][:, bass.ts(n, CS)], p_xi[:],
                                     AF.Silu,
                                     bias=convb_t[:, li : li + 1], scale=1.0)
                p_z = psum.tile([DI, CS], F32, tag="mm512")
                MM(p_z[:], ipz_t[:, li * DI : (li + 1) * DI],
                   t16[:, 4 + n * CS : 4 + (n + 1) * CS],
                   start=True, stop=True)
                nc.scalar.activation(ly["sz"][:, bass.ts(n, CS)], p_z[:],
                                     AF.Silu)

            def chunk2(n, ev):
                p_d = psum.tile([DI, CS], F32, tag="mm512")
                MM(p_d[:], wd_t[:, li * DI : (li + 1) * DI],
                   ly["xi_c"][:, bass.ts(n, CS)], start=True, stop=True)
                # ev = exp(p_d + dt_b); delta = ln(1 + ev) applied in s_half.
                # Exp/Ln share the natural_log_exp act table -> no reloads.
                nc.scalar.activation(ev[:, bass.ts(n % 4, CS)], p_d[:],
                                     AF.Exp,
                                     bias=dtb_t[:, li : li + 1], scale=1.0)
                p_bc = psum.tile([2 * DSL, CS], F32, tag="mm512")
                MM(p_bc[:], bct_t[:, li * 2 * DSL : (li + 1) * 2 * DSL],
                   ly["xi_c"][:, bass.ts(n, CS)], start=True, stop=True)
                nc.scalar.activation(ly["bc_sb"][:, bass.ts(n, CS)], p_bc[:],
                                     AF.Copy)

            def half_chunks(hf):
                for n in range(hf * 4, hf * 4 + 4):
                    chunk1(n)
                ev = tail.tile([DI, LH], F32, tag=f"ev{hf}")
                for n in range(hf * 4, hf * 4 + 4):
                    chunk2(n, ev)
                sl = slice(hf * LH, (hf + 1) * LH)
                nc.sync.dma_start(bc_dram[li][:, sl], ly["bc_sb"][:, sl])
                return ev

            def s_half(hf, ev, inject=None):
                sl = slice(hf * LH, (hf + 1) * LH)
                dlt, du, xi_c = ly["dlt"], ly["du"], ly["xi_c"]
                nc.scalar.activation(dlt[:, sl], ev[:], AF.Ln, bias=1.0,
                                     scale=1.0)
                nc.vector.tensor_tensor(du[:, sl], dlt[:, sl], xi_c[:, sl],
                                        OP.mult)
                acc = None
                pend = None
                for s in range(DSL):
                    if s == INJ and inject is not None:
                        inject()
                    dA = sloop.tile([DI, LH], FP16, tag="dA")
                    nc.scalar.activation(
                        dA[:], dlt[:, sl], AF.Exp,
                        scale=acols_t[:, li * DSL + s : li * DSL + s + 1])
                    brep = sloop.tile([DI, LH], BF16, tag="brep")
                    nc.sync.dma_start(
                        brep[:],
                        bc_dram[li][s : s + 1, sl].broadcast_to((DI, LH)))
                    crep = sloop.tile([DI, LH], BF16, tag="crep")
                    nc.sync.dma_start(
                        crep[:],
                        bc_dram[li][DSL + s : DSL + s + 1, sl].broadcast_to(
                            (DI, LH)))
                    xs = sloop.tile([DI, LH], BF16, tag="xs")
                    nc.vector.tensor_tensor(xs[:], du[:, sl], brep[:], OP.mult)
                    hs = sloop.tile([DI, LH], BF16, tag="hs")
                    init = 0.0 if hf == 0 else hmid[:, s : s + 1]
                    nc.vector.tensor_tensor_scan(hs[:], dA[:], xs[:], init,
                                                 OP.mult, OP.add)
                    if hf == 0:
                        nc.vector.tensor_copy(hmid[:, s : s + 1],
                                              hs[:, LH - 1 : LH])
                    if pend is not None:
                        anew = sloop.tile([DI, LH], BF16, tag=f"acc{hf}")
                        if acc is None:
                            # fold the D*u term into the first accumulate
                            nc.vector.scalar_tensor_tensor(
                                anew[:], xi_c[:, sl],
                                dcol_t[:, li : li + 1], pend[:],
                                OP.mult, OP.add)
                        else:
                            nc.vector.tensor_tensor(anew[:], acc[:], pend[:],
                                                    OP.add)
                        acc = anew
                    hc = sloop.tile([DI, LH], BF16, tag="hc")
                    nc.vector.tensor_tensor(hc[:], hs[:], crep[:], OP.mult)
                    pend = hc
                anew = sloop.tile([DI, LH], BF16, tag=f"acc{hf}")
                nc.vector.tensor_tensor(anew[:], acc[:], pend[:], OP.add)
                return anew

            def half_tail(hf, acc):
                sl = slice(hf * LH, (hf + 1) * LH)
                yg = tail.tile([DI, LH], BF16, tag=f"yg{hf}")
                nc.vector.tensor_tensor(yg[:], acc[:], ly["sz"][:, sl],
                                        OP.mult)
                dt_part = tail.tile([HD, LH], BF16, tag=f"dtp{hf}")
                for n in range(4):
                    p_o = psum.tile([HD, CS], F32, tag="mm512")
                    MM(p_o[:], opt_t[:, li * HD : (li + 1) * HD],
                       yg[:, bass.ts(n, CS)], start=True, stop=True)
                    nc.scalar.activation(dt_part[:, bass.ts(n, CS)], p_o[:],
                                         AF.Copy)
                nc.sync.dma_start(y_in[hf][:], dt_part[:])
                nc.gpsimd.collective_compute(
                    "AllReduce", OP.add, replica_groups=PAIRS,
                    ins=[y_in[hf][:]], outs=[y_out[hf][:]])

            def half_update(hf):
                sl = slice(4 + hf * LH, 4 + (hf + 1) * LH)
                # reuse dtp slot: its DMA into y_in completes before the
                # collective can produce y_out
                dtsum = tail.tile([HD, LH], BF16, tag=f"dtp{hf}")
                nc.sync.dma_start(dtsum[:], y_out[hf][:])
                nc.vector.tensor_tensor(t_t[:, sl], t_t[:, sl], dtsum[:],
                                        OP.add)
                if li < NL - 1:
                    nc.vector.tensor_copy(t16[:, sl], t_t[:, sl])

            ly.update(chunk1=chunk1, chunk2=chunk2, half_chunks=half_chunks,
                      s_half=s_half, half_tail=half_tail,
                      half_update=half_update)
            return ly

        cur = mk_layer(0)
        ev0 = cur["half_chunks"](0)
        pend_update = None
        for li in range(NL):
            acc0 = cur["s_half"](0, ev0, inject=pend_update)
            ev1 = cur["half_chunks"](1)
            cur["half_tail"](0, acc0)
            hu = cur["half_update"]
            acc1 = cur["s_half"](1, ev1, inject=lambda hu=hu: hu(0))
            if li + 1 < NL:
                nxt = mk_layer(li + 1)
                ev0 = nxt["half_chunks"](0)
            cur["half_tail"](1, acc1)
            pend_update = lambda hu=hu: hu(1)
            if li + 1 < NL:
                cur = nxt

        # ---------------- decoder ----------------
        d1_taps = const.tile([HD, 9 * HD], FP16)
        nc.sync.dma_start(d1_taps[:], dec1_tap[:])
        d2_taps = const.tile([HD, 9 * NFH], FP16)
        nc.sync.dma_start(d2_taps[:], dec2_tap[:])
        d1g_t = const.tile([HD, 1], F32)
        nc.sync.dma_start(d1g_t[:], dec1_g[:])
        d1be_t = const.tile([HD, 1], F32)
        nc.sync.dma_start(d1be_t[:], dec1_be[:])
        d2b_t = const.tile([NFH, 1], F32)
        nc.sync.dma_start(d2b_t[:], dec2_b[:])
        padA = persist.tile([HD, PADL], FP16)
        nc.vector.memset(padA[:], 0.0)
        padB = persist.tile([HD, PADL], FP16)
        nc.vector.memset(padB[:], 0.0)
        out_pad = persist.tile([NFH, PADL], BF16)

        def interior(tile_ap):
            return tile_ap[:, PBASE : PBASE + PW * H].rearrange(
                "p (h w) -> p h w", w=PW)[:, :, 0:W]

        # top half of the image is final before the last half-1 residual
        # update; start dec1 on it while the last y-collective completes
        nc.scalar.activation(
            interior(padA)[:, 0:32, :],
            t_t[:, 4 : 4 + LH].rearrange("p (h w) -> p h w", w=W),
            AF.Copy)

        def conv9(dst_tile, src_tile, taps_tile, m_out, tapw, evict,
                  n0=0, n1=None):
            total = PW * H
            nch = (total + CS - 1) // CS
            for n in range(n0, nch if n1 is None else n1):
                c0 = PBASE + n * CS
                cw = min(CS, PBASE + total - c0)
                pt = psum.tile([m_out, CS], F32, tag="mm512")
                for ti in range(9):
                    dy, dx = ti // 3, ti % 3
                    off = c0 + (dy - 1) * PW + (dx - 1)
                    MM(pt[:, 0:cw],
                       taps_tile[:, ti * tapw : ti * tapw + m_out],
                       src_tile[:, off : off + cw],
                       start=(ti == 0), stop=(ti == 8))
                evict(dst_tile[0:m_out, c0 : c0 + cw], pt[:, 0:cw])

        d1_evict = lambda d, p: nc.scalar.activation(d, p, AF.Copy)
        conv9(padB, padA, d1_taps, HD, HD, d1_evict, n0=0, n1=3)
        pend_update()       # last layer's half-1 residual into t_t
        nc.scalar.activation(
            interior(padA)[:, 32:64, :],
            t_t[:, 4 + LH :].rearrange("p (h w) -> p h w", w=W),
            AF.Copy)
        conv9(padB, padA, d1_taps, HD, HD, d1_evict, n0=3)

        d1_int = interior(padB)
        ds1 = small.tile([HD, 1], F32, tag="ds1")
        nc.vector.tensor_reduce(ds1[:], d1_int, axis=X.XY, op=OP.add)
        ds2 = small.tile([HD, 1], F32, tag="ds2")
        nc.scalar.activation(interior(padA), d1_int, AF.Square,
                             accum_out=ds2[:])
        packed = small.tile([HD, 2], F32, tag="pk")
        nc.vector.tensor_copy(packed[:, 0:1], ds1[:])
        nc.vector.tensor_copy(packed[:, 1:2], ds2[:])
        nc.sync.dma_start(cc_in[:], packed[:])
        # 4-way: each group holds all 4 batches exactly once -> exact stats
        nc.gpsimd.collective_compute(
            "AllReduce", OP.add,
            replica_groups=[[0, 1, 2, 3], [4, 5, 6, 7]],
            ins=[cc_in[:]], outs=[cc_out[:]])
        red = small.tile([HD, 2], F32, tag="red")
        nc.sync.dma_start(red[:], cc_out[:])
        sc1, bi1 = bn_scale_bias(red[:, 0:1], red[:, 1:2], B * L,
                                 d1g_t[:], d1be_t[:], "bn1")

        # h2 into padA (pads still zero outside interior; square scratch
        # gets overwritten)
        nc.scalar.activation(interior(padA), d1_int, AF.Relu,
                             bias=bi1[:], scale=sc1[:])
        conv9(out_pad, padA, d2_taps, NFH, NFH,
              lambda d, p: nc.scalar.activation(
                  d, p, AF.Identity, bias=d2b_t[:], scale=1.0))
        out_int = out_pad[:NFH, PBASE : PBASE + PW * H].rearrange(
            "p (h w) -> p h w", w=PW)[:, :, 0:W]
        nc.sync.dma_start(out_ext[:].rearrange("p (h w) -> p h w", w=W),
                          out_int)

    split_excess_waits(nc)
    return nc


_CACHED = {}


def _get_kernel():
    if "nc" not in _CACHED:
        _CACHED["nc"] = build_kernel()
    return _CACHED["nc"]


def _host_inputs(inputs):
    f32 = np.float32
    bf16 = ml_dtypes.bfloat16
    x = np.asarray(inputs["x"], f32)
    enc_w = np.asarray(inputs["enc_w"], f32)
    in_proj = np.asarray(inputs["in_proj"], f32)
    conv_w = np.asarray(inputs["conv_w"], f32)
    x_proj = np.asarray(inputs["x_proj"], f32)
    dt_w = np.asarray(inputs["dt_w"], f32)
    A_log = np.asarray(inputs["A_log"], f32)
    out_proj = np.asarray(inputs["out_proj"], f32)
    dec1_w = np.asarray(inputs["dec1_w"], f32)
    dec2_w = np.asarray(inputs["dec2_w"], f32)

    xp = np.zeros((B, NB, H + 2, W + 2), f32)
    xp[:, :, 1:-1, 1:-1] = x
    cols = np.empty((NB, 3, 3, B, L), f32)
    for dy in range(3):
        for dx in range(3):
            cols[:, dy, dx] = (
                xp[:, :, dy : dy + H, dx : dx + W]
                .reshape(B, NB, L).transpose(1, 0, 2))
    cols_b = cols.reshape(45, B, L)
    enc_w2 = np.ascontiguousarray(enc_w.reshape(HD, 45).T)

    ip_tap = np.empty((HD, NL, DC, DI), f32)
    ip_z = np.empty((HD, NL, DI), f32)
    wd_T = np.empty((DI, NL, DI), f32)
    bc_full = np.empty((DI, NL, 2 * DS), f32)
    a_full = np.empty((DI, NL, DS), f32)
    op_T = np.empty((DI, NL, HD), f32)
    for i in range(NL):
        for k in range(DC):
            ip_tap[:, i, k, :] = (conv_w[i][:, k : k + 1] * in_proj[i][:DI]).T
        ip_z[:, i, :] = in_proj[i][DI:].T
        wd_T[:, i, :] = (dt_w[i] @ x_proj[i][:DTR]).T
        bc_full[:, i, :] = x_proj[i][DTR:].T
        a_full[:, i, :] = -np.exp(A_log[i])   # dA = exp(a_col * delta)
        op_T[:, i, :] = out_proj[i].T

    dec1_tap = np.empty((HD, 9, HD), f32)
    dec2_tap = np.empty((HD, 9, NF), f32)
    for ti in range(9):
        dy, dx = ti // 3, ti % 3
        dec1_tap[:, ti, :] = dec1_w[:, :, dy, dx].T
        dec2_tap[:, ti, :] = dec2_w[:, :, dy, dx].T
    dec2_b_full = np.asarray(inputs["dec2_b"], f32)

    common = {
        "enc_w2": enc_w2.astype(np.float16),
        "enc_g": np.asarray(inputs["enc_g"], f32).reshape(HD, 1),
        "enc_be": np.asarray(inputs["enc_be"], f32).reshape(HD, 1),
        "ip_tap": ip_tap.reshape(HD, NL * DC * DI).astype(np.float16),
        "ip_z": ip_z.reshape(HD, NL * DI).astype(np.float16),
        "conv_b": np.ascontiguousarray(
            np.asarray(inputs["conv_b"], f32).T),           # (DI, NL)
        "wd_T": wd_T.reshape(DI, NL * DI).astype(bf16),

        "dt_b": np.ascontiguousarray(np.asarray(inputs["dt_b"], f32).T),

        "d_col": np.ascontiguousarray(np.asarray(inputs["Dp"], f32).T) / 2.0,
        "op_T": op_T.reshape(DI, NL * HD).astype(bf16),
        "dec1_tap": dec1_tap.reshape(HD, 9 * HD).astype(np.float16),
        "dec1_g": np.asarray(inputs["dec1_g"], f32).reshape(HD, 1),
        "dec1_be": np.asarray(inputs["dec1_be"], f32).reshape(HD, 1),
    }
    in_maps = []
    for c in range(8):
        b0 = c % B
        sr = (c // B) * DSL
        ch0 = (c // B) * NFH
        m = dict(common)
        order = [b0] + [bb for bb in range(B) if bb != b0]
        m["enc_im2col"] = np.ascontiguousarray(
            cols_b[:, order, :].reshape(45, B * L)).astype(np.float16)
        bcs = np.concatenate(
            [bc_full[:, :, sr : sr + DSL],
             bc_full[:, :, DS + sr : DS + sr + DSL]], axis=2)
        m["bc_T"] = np.ascontiguousarray(
            bcs.reshape(DI, NL * 2 * DSL)).astype(bf16)
        m["a_cols"] = np.ascontiguousarray(
            a_full[:, :, sr : sr + DSL].reshape(DI, NL * DSL))
        m["dec2_tap"] = np.ascontiguousarray(
            dec2_tap[:, :, ch0 : ch0 + NFH].reshape(HD, 9 * NFH)
        ).astype(np.float16)
        m["dec2_b"] = np.ascontiguousarray(
            dec2_b_full[ch0 : ch0 + NFH].reshape(NFH, 1))
        in_maps.append(m)
    return in_maps


def kernel(**inputs):
    nc = _get_kernel()
    in_maps = _host_inputs(inputs)
    res = run_bass_kernel_spmd(nc, in_maps, core_ids=list(range(8)))
    out = np.empty((B, NF, H, W), np.float32)
    for b_ in range(B):
        out[b_, :NFH] = np.asarray(
            res.results[b_]["out"], np.float32).reshape(NFH, H, W)
        out[b_, NFH:] = np.asarray(
            res.results[b_ + B]["out"], np.float32).reshape(NFH, H, W)
    return out


if __name__ == "__main__":
    sys.path.insert(0, "/root/problem")
    import reference as ref

    inp = {k: np.asarray(v) for k, v in ref.setup_inputs().items()}
    got = kernel(**inp)
    print("kernel ran, output shape:", got.shape)
